# revision 27
# baseline (speedup 1.0000x reference)
"""Trainium2 Bass kernel for nn_Decoder (dense_transformer) — v2.

Key restructuring vs v1 baseline (1532us):
  - Attention matmuls are PE-efficient: scores stream nrm_T (N=512) against a
    32-col stationary U' slice, 4 batch elements packed into one PSUM bank via
    tile_position col-tiling; values stream nrm_nat (N=129) against 32-col
    transposed-softmax weights. No more 128-col weight reloads per (b,chunk).
  - The softmax denominator comes free from an appended ones-column in the
    values rhs (col 128), so rn = ps[:,0:128] * recip(ps[:,128]).
  - All transposes are regular matmuls with a bf16 identity rhs (keeps the PE
    HAM-warm at 2.4GHz, unlike transpose-mode).
  - enc_out is cast fp32->bf16 in the DMA (SWDGE), halving DVE stats/apply
    cost; bn_stats runs per-b (4 chunks in one op) and the mean/var
    aggregation is done with a handful of [128,64] strided ops per group.
  - Layer weights are loaded + folded ONCE (not per supergroup).

Sharding: pure data parallel, batch 2048 -> 8 cores x 256.
"""

import math
from contextlib import ExitStack

import numpy as np

import concourse.bass as bass
import concourse.tile as tile
from concourse import bacc, mybir
from concourse.bass_utils import run_bass_kernel_spmd
from concourse.masks import make_identity

F32 = mybir.dt.float32
BF16 = mybir.dt.bfloat16
I32 = mybir.dt.int32
AF = mybir.ActivationFunctionType
OP = mybir.AluOpType

RSQRT_MAGIC_H = 0x5EF759DF  # quake magic 0x5f3759df shifted for vh = v/2 seed

P = 128
T = 512
E = 128
H = 8
D = 16
L = 3
NCH = T // P            # 4 t-chunks
BN_S = 1.0 / math.sqrt(1.0 + 1e-5)
EPS = 1e-5
N_CORES = 8
B_FULL = 2048
SG = 32                 # supergroup batch size (residual-stream width)
# softmax denominators come from the exp activation's accum_out, so the
# values rhs is just the E normalized columns (no appended ones column).

SHARDED = ("enc_out", "x1", "x2", "x3")


def _ap(t, offset, pattern):
    return bass.AP(tensor=t.tensor, offset=offset, ap=[list(p) for p in pattern])


def tap(ap, extra_off, free_pattern):
    """Sub-AP of a tile AP: keep partition dim, replace free dims."""
    return bass.AP(tensor=ap.tensor, offset=ap.offset + extra_off,
                   ap=[list(ap.ap[0])] + [list(p) for p in free_pattern])


def statenet(ctx, tc, ins, q_T, BC, p_a, p_b):
    """Conv/FC front-end producing q0 [E, BC] into q_T. (v1 logic verbatim.)"""
    nc = tc.nc
    dma = nc.sync.dma_start
    NB = (BC + P - 1) // P

    def psum(pool, shape, dt_=F32):
        return pool.tile(shape, dt_, tag=pool.name, name=pool.name + "_t")

    id_f32 = ctx._id_f32

    with tc.tile_pool(name="snet", bufs=1) as sn:
        x1T = sn.tile([111, BC], F32)
        x2T = sn.tile([28, BC], F32)
        cat64 = sn.tile([64, BC], F32)
        cat16 = sn.tile([16, BC], F32)
        x3c = sn.tile([4, BC], F32)
        x3T = x3c[0:4, :]
        x1_f = ins["x1"].rearrange("b c h w -> b (c h w)")
        x2_f = ins["x2"].rearrange("b c h w -> b (c h w)")
        for i in range(NB):
            n = min(P, BC - i * P)
            for (srcx, dstT, w) in ((x1_f, x1T[:], 111), (x2_f, x2T[:], 28),
                                    (ins["x3"], x3T, 4)):
                xin = sn.tile([P, w], F32, tag="xin")
                dma(out=xin[:n, :], in_=srcx[i * P:i * P + n, :])
                pst = psum(p_a, [w, P])
                nc.tensor.transpose(pst[:, :n], xin[:n, :], id_f32[:n, :n])
                nc.scalar.copy(dstT[:, i * P:i * P + n], pst[:, :n])

        def conv_w(dram_ap, O_, C_, gname, bname, cbname):
            KK = C_ * 3
            ws = sn.tile([O_, KK], F32, tag="ws" + gname)
            dma(out=ws[:], in_=_ap(dram_ap, 1, [[C_ * 9, O_], [9, C_], [3, 3]]))
            g = sn.tile([O_, 1], F32, tag="g" + gname)
            dma(out=g[:], in_=ins[gname])
            gp = sn.tile([O_, 1], F32, tag="gp" + gname)
            nc.scalar.mul(gp[:], g[:], BN_S)
            cb = sn.tile([O_, 1], F32, tag="cb" + gname)
            dma(out=cb[:], in_=ins[cbname])
            bb = sn.tile([O_, 1], F32, tag="bb" + gname)
            dma(out=bb[:], in_=ins[bname])
            beff = sn.tile([O_, 1], F32, tag="be" + gname)
            nc.vector.tensor_mul(beff[:], cb[:], gp[:])
            nc.vector.tensor_add(beff[:], beff[:], bb[:])
            wsc = sn.tile([O_, KK], F32, tag="wsc" + gname)
            nc.vector.tensor_scalar_mul(wsc[:], ws[:], gp[:])
            pswt = psum(p_a, [KK, O_])
            nc.tensor.transpose(pswt[:], wsc[:], id_f32[:O_, :O_])
            wT = sn.tile([KK, O_], F32, tag="wT" + gname)
            nc.scalar.copy(wT[:], pswt[:])
            return wT, beff

        w1T, b1e = conv_w(ins["c11_w"], 8, 3, "bn11_g", "bn11_b", "c11_b")
        w2T, b2e = conv_w(ins["c12_w"], 8, 8, "bn12_g", "bn12_b", "c12_b")
        w3T, b3e = conv_w(ins["c21_w"], 8, 7, "bn21_g", "bn21_b", "c21_b")

        def im2col(srcT, C_, W_):
            rhs = sn.tile([24, 37, BC], F32, tag="im", name="imt")[:C_ * 3, :W_, :]
            nc.vector.memset(rhs[:], 0.0)
            for c in range(C_):
                for kh in range(3):
                    lo = max(0, 1 - kh)
                    hi = min(W_, W_ + 1 - kh)
                    n = hi - lo
                    s0 = c * W_ + lo + kh - 1
                    k_ = c * 3 + kh
                    dma(out=rhs[k_:k_ + 1, lo:hi, :], in_=srcT[s0:s0 + n, :])
            return rhs

        def conv_apply(rhs, wT, beff, O_, W_):
            y = sn.tile([8, 37, BC], F32, tag="yt", name="ytt")[:O_, :W_, :]
            step = max(1, 512 // BC)
            for i0 in range(0, W_, step):
                n = min(step, W_ - i0)
                psc = psum(p_b, [O_, step, BC])
                nc.tensor.matmul(psc[:, :n, :], wT[:], rhs[:, i0:i0 + n, :])
                nc.scalar.activation(y[:, i0:i0 + n, :], psc[:, :n, :],
                                     AF.Relu, bias=beff[:])
            return y

        r9 = im2col(x1T, 3, 37)
        y1 = conv_apply(r9, w1T, b1e, 8, 37)
        r24 = sn.tile([24, 37, BC], F32, tag="im", name="imt")
        nc.vector.memset(r24[:], 0.0)
        for c in range(8):
            for kh in range(3):
                lo = max(0, 1 - kh)
                hi = min(37, 37 + 1 - kh)
                n = hi - lo
                k_ = c * 3 + kh
                dma(out=r24[k_:k_ + 1, lo:hi, :],
                    in_=y1[c:c + 1, lo + kh - 1:lo + kh - 1 + n, :])
        y2 = conv_apply(r24, w2T, b2e, 8, 37)

        r21 = im2col(x2T, 7, 4)
        y2b = conv_apply(r21, w3T, b3e, 8, 4)

        y2r = []
        for k, (ilo, ihi) in enumerate(((0, 16), (16, 32), (32, 37))):
            ni = ihi - ilo
            t_ = sn.tile([ni * 8, BC], F32, tag=f"y2r{k}")
            for o in range(8):
                dma(out=t_[o * ni:(o + 1) * ni, :], in_=y2[o:o + 1, ilo:ihi, :])
            y2r.append(t_)
        y2br = sn.tile([32, BC], F32)
        for o in range(8):
            dma(out=y2br[o * 4:(o + 1) * 4, :], in_=y2b[o:o + 1, :, :])

        ps_h1 = psum(p_b, [64, BC])
        for k, (ilo, ihi) in enumerate(((0, 16), (16, 32), (32, 37))):
            ni = ihi - ilo
            fw = sn.tile([ni * 8, 64], F32, tag=f"fw{k}")
            dma(out=fw[:], in_=_ap(ins["fc1_w"], ilo * 64,
                                   [[37 * 64, 8], [64, ni], [1, 64]]))
            nc.tensor.matmul(ps_h1[:], fw[:], y2r[k][:],
                             start=(k == 0), stop=(k == 2))
        fb1 = sn.tile([64, 1], F32)
        dma(out=fb1[:], in_=ins["fc1_b"])
        h1 = cat64[0:64, :]
        nc.scalar.activation(h1, ps_h1[:], AF.Relu, bias=fb1[:])

        fw2 = sn.tile([32, 16], F32)
        dma(out=fw2[:], in_=ins["fc2_w"])
        ps_h2 = psum(p_b, [16, BC])
        nc.tensor.matmul(ps_h2[:], fw2[:], y2br[:])
        fb2 = sn.tile([16, 1], F32)
        dma(out=fb2[:], in_=ins["fc2_b"])
        h2 = cat16[0:16, :]
        nc.scalar.activation(h2, ps_h2[:], AF.Relu, bias=fb2[:])

        fcw64 = sn.tile([64, E], F32)
        dma(out=fcw64[:], in_=ins["fc_w"][0:64, :])
        fcw16 = sn.tile([16, E], F32)
        dma(out=fcw16[:], in_=ins["fc_w"][64:80, :])
        fcw3 = sn.tile([4, E], F32)
        dma(out=fcw3[:], in_=ins["fc_w"][80:84, :])
        ps_q0 = psum(p_b, [P, BC])
        nc.tensor.matmul(ps_q0[:], fcw64[:], cat64[:], start=True, stop=False)
        nc.tensor.matmul(ps_q0[:], fcw16[:], cat16[:], start=False, stop=False)
        nc.tensor.matmul(ps_q0[:], fcw3[:], x3c[:], start=False, stop=True)
        fcb = sn.tile([P, 1], F32)
        dma(out=fcb[:], in_=ins["fc_b"])
        nc.scalar.activation(q_T[:], ps_q0[:], AF.Relu, bias=fcb[:])


def decoder_body(ctx: ExitStack, tc: tile.TileContext, out_ap: bass.AP,
                 ins: dict, BC: int, stage: int = 99):
    nc = tc.nc
    dma = nc.sync.dma_start
    NSG = BC // SG

    def dbg_out(tag_ap):
        nc.sync.dma_start(out=out_ap.rearrange("b o -> o b"),
                          in_=tag_ap[0:37, 0:BC])

    # ---------------- pools ----------------
    const = ctx.enter_context(tc.tile_pool(name="const", bufs=1))
    wts = ctx.enter_context(tc.tile_pool(name="wts", bufs=1))
    perm = ctx.enter_context(tc.tile_pool(name="perm", bufs=1))
    # PSUM: 8 banks; each pool = bufs x max-2KB tile
    p_sc = ctx.enter_context(tc.tile_pool(name="p_sc", bufs=2, space="PSUM"))
    p_tr = ctx.enter_context(tc.tile_pool(name="p_tr", bufs=2, space="PSUM"))
    p_wt = ctx.enter_context(tc.tile_pool(name="p_wt", bufs=2, space="PSUM"))
    p_sm = ctx.enter_context(tc.tile_pool(name="p_sm", bufs=2, space="PSUM"))

    def psum(pool, shape, dt_=F32):
        return pool.tile(shape, dt_, tag=pool.name, name=pool.name + "_t")

    id_f32 = const.tile([P, P], F32)
    id_bf = const.tile([P, P], BF16)
    make_identity(nc, id_f32[:])
    make_identity(nc, id_bf[:])
    ctx._id_f32 = id_f32
    ones_col = const.tile([P, 1], F32)
    nc.vector.memset(ones_col[:], 1.0)
    ones_row = const.tile([1, P], F32)
    nc.vector.memset(ones_row[:], 1.0)
    # Newton-rsqrt constants (all-DVE rstd; keeps Sqrt off ScalarE so the
    # activation table never swaps away from exp_and_others). Single tuned
    # Newton step y0*(A - B*vh*y0^2): max rel err 8.8e-4.
    magic_t = const.tile([P, BC], I32)
    nc.vector.memset(magic_t[:], RSQRT_MAGIC_H)
    ca_t = const.tile([P, BC], F32)
    nc.vector.memset(ca_t[:], 1.50133365)
    # ind8[h, h'*32+j] = (h == h'): K=8 indicator used to add per-head/-chunk
    # biases with a single accumulating matmul (rows 0:4, cols 0:128 double as
    # the K=4 FFN-bias indicator).
    ind8 = const.tile([8, 8, SG], BF16)
    ones_row_bf = const.tile([1, SG], BF16)
    nc.vector.memset(ones_row_bf[:], 1.0)
    nc.vector.memset(ind8[:], 0.0)
    for hh in range(8):
        nc.sync.dma_start(out=ind8[hh:hh + 1, hh, :], in_=ones_row_bf[:])

    def rsqrt_nr(vh_sl, p_, n_, tagp, out_sl=None):
        """rstd = 1/sqrt(2*vh) via bit-trick seed + 2 Newton iters (DVE only).

        vh_sl: [p_, n_] f32 AP holding (var + eps) / 2. Writes into out_sl
        if given (returns it), else into a scratch tile."""
        ti = work.tile([p_, n_], I32, tag=tagp + "ti", name=tagp + "ti")
        y = work.tile([p_, n_], F32, tag=tagp + "y", name=tagp + "y")
        t = work.tile([p_, n_], F32, tag=tagp + "t", name=tagp + "t")
        nc.vector.tensor_scalar(ti[:], vh_sl.bitcast(I32), 1, None,
                                op0=OP.logical_shift_right)
        nc.vector.tensor_tensor(y[:].bitcast(I32), magic_t[0:p_, 0:n_],
                                ti[:], op=OP.subtract)
        nc.vector.tensor_mul(t[:], y[:], y[:])
        nc.vector.tensor_tensor(t[:], vh_sl, t[:], op=OP.mult)
        nc.vector.scalar_tensor_tensor(t[:], t[:], -1.00091486,
                                       ca_t[0:p_, 0:n_],
                                       op0=OP.mult, op1=OP.add)
        dst = y[:] if out_sl is None else out_sl
        nc.vector.tensor_tensor(dst, y[:], t[:], op=OP.mult)
        return y[:] if out_sl is None else out_sl

    q_T = perm.tile([P, BC], F32)            # persistent residual [E, b]

    # =======================================================================
    # StateNet (scoped; its SBUF is reclaimed before the big pools open)
    # =======================================================================
    statenet(ctx, tc, ins, q_T, BC, p_sm, p_sc)
    if stage <= 1:
        dbg_out(q_T)
        return

    # =======================================================================
    # Phase 0: load + fold all layer weights once
    # =======================================================================
    def load_col(name, l, n, tg):
        t_ = wts.tile([n, 1], F32, tag=tg)
        src = ins[name]
        dma(out=t_[:], in_=src[l] if l is not None else src)
        return t_

    W = []  # per-layer dict of folded weights (raw loads live in a scope)
    with tc.tile_pool(name="wraw", bufs=1) as wr:
        for l in range(L):
            w = {}
            wq_t = wr.tile([E, E], F32, tag="wq")
            dma(out=wq_t[:], in_=ins["wq"][l])
            wk_t = wr.tile([E, E], F32, tag="wk", name="wk_t")
            dma(out=wk_t[:], in_=ins["wk"][l])
            pj_t = wr.tile([E, E], F32, tag="pj", name="pj_t")
            dma(out=pj_t[:], in_=ins["proj_w"][l])
            wv_t = wr.tile([E, E], F32, tag="wv", name="wv_t")
            dma(out=wv_t[:], in_=ins["wv"][l])
            g1 = load_col("ln1_g", l, P, f"g1{l}")
            g2 = wr.tile([P, 1], F32, tag="g2")
            dma(out=g2[:], in_=ins["ln2_g"][l])
            b2 = wr.tile([P, 1], F32, tag="b2")
            dma(out=b2[:], in_=ins["ln2_b"][l])
            g3 = wr.tile([P, 1], F32, tag="g3")
            dma(out=g3[:], in_=ins["ln3_g"][l])
            b3 = load_col("ln3_b", l, P, f"b3{l}")
            g4 = wr.tile([P, 1], F32, tag="g4")
            dma(out=g4[:], in_=ins["ln4_g"][l])
            b4 = load_col("ln4_b", l, P, f"b4{l}")
            pjb = wr.tile([P, 1], F32, tag="pjb")
            dma(out=pjb[:], in_=ins["proj_b"][l])
            w["g1"], w["b3"], w["b4"] = g1, b3, b4
            w["fb2"] = load_col("ff_b2", l, P, f"fb2{l}")

            wq_e = wts.tile([E, E], BF16, tag=f"wqe{l}")
            nc.vector.tensor_scalar_mul(wq_e[:], wq_t[:], g3[:])
            w["wq_e"] = wq_e
            qb_ps = psum(p_sm, [16, H])
            for h in range(H):
                nc.tensor.matmul(qb_ps[:, h:h + 1],
                                 wq_t[:, 16 * h:16 * h + 16], b3[:])
            qb_spl = wr.tile([16, H], F32, tag="qb", name="qb_spl")
            nc.scalar.copy(qb_spl[:], qb_ps[:])
            ps_qbT = psum(p_sm, [H, 16])
            nc.tensor.matmul(ps_qbT[:], qb_spl[:], id_f32[0:16, 0:16])
            qbT = wts.tile([H, 16], BF16, tag=f"qbT{l}")
            nc.scalar.copy(qbT[:], ps_qbT[:])
            w["qbT"] = qbT

            wk_spl = wts.tile([16, H, E], BF16, tag=f"wks{l}")
            for hh in range(2):
                ps_kT = psum(p_sm, [16, 4, E])
                for h4 in range(4):
                    h = hh * 4 + h4
                    nc.tensor.transpose(ps_kT[:, h4, :],
                                        wk_t[:, 16 * h:16 * h + 16], id_f32[:])
                nc.scalar.copy(wk_spl[:, 4 * hh:4 * hh + 4, :], ps_kT[:])
            w["wk_spl"] = wk_spl

            wv_e = wr.tile([E, E], F32, tag="wve", name="wv_e")
            nc.vector.tensor_scalar_mul(wv_e[:], wv_t[:], g2[:])
            wv_bf = wts.tile([E, E], BF16, tag=f"wvbf{l}")
            nc.vector.tensor_copy(wv_bf[:], wv_e[:])
            w["wv_bf"] = wv_bf
            ps2 = psum(p_sm, [P, 1])
            nc.tensor.matmul(ps2[:], wv_e[:], b2[:])
            c2 = wr.tile([P, 1], F32, tag="c2", name="c2")
            nc.scalar.copy(c2[:], ps2[:])
            ps2b = psum(p_sm, [P, 1])
            nc.tensor.matmul(ps2b[:], pj_t[:], c2[:])
            bias2 = wts.tile([P, 1], F32, tag=f"bias2{l}")
            nc.vector.tensor_add(bias2[:], ps2b[:], pjb[:])
            w["bias2"] = bias2

            pj_bf = wts.tile([16, H, E], BF16, tag=f"pjs{l}")
            pj_f = wr.tile([16, H, E], F32, tag="pjf", name="pj_f")
            dma(out=pj_f[:], in_=_ap(ins["proj_w"], l * E * E,
                                     [[E, 16], [16 * E, H], [1, E]]))
            nc.vector.tensor_copy(pj_bf[:], pj_f[:])
            w["pj_bf"] = pj_bf

            f1_t = wr.tile([E, 4 * E], F32, tag="f1", name="f1_t")
            dma(out=f1_t[:], in_=ins["ff_w1"][l])
            f1_e = wts.tile([E, 4 * E], BF16, tag=f"f1e{l}")
            f1_ef = wr.tile([E, 4 * E], F32, tag="f1ef", name="f1_ef")
            nc.vector.tensor_scalar_mul(f1_ef[:], f1_t[:], g4[:])
            nc.vector.tensor_copy(f1_e[:], f1_ef[:])
            w["f1_e"] = f1_e
            ps3 = psum(p_sm, [P, 4])
            for m in range(4):
                nc.tensor.matmul(ps3[:, m:m + 1], f1_ef[:, m * E:(m + 1) * E],
                                 b4[:])
            fb1_ = wr.tile([P, 4], F32, tag="fb1", name="fb1_")
            dma(out=fb1_[:], in_=ins["ff_b1"][l].rearrange("(c p) -> p c", p=P))
            fb1e = wr.tile([P, 4], F32, tag="fb1e", name="fb1e")
            nc.vector.tensor_add(fb1e[:], ps3[:], fb1_[:])
            ps_bT = psum(p_sm, [4, P])
            nc.tensor.matmul(ps_bT[:], fb1e[:], id_f32[:])
            fb1eT = wts.tile([4, P], BF16, tag=f"fb1eT{l}")
            nc.scalar.copy(fb1eT[:], ps_bT[:])
            w["fb1eT"] = fb1eT

            f2_f = wr.tile([P, 4, E], F32, tag="f2f", name="f2_f")
            dma(out=f2_f[:],
                in_=ins["ff_w2"][l].rearrange("(c p) e -> p c e", p=P))
            # halved so gelu = (f2/2)@g + (f2/2)@(g*tanh) needs no +1/×0.5 ops
            f2_bf = wts.tile([P, 4, E], BF16, tag=f"f2{l}")
            nc.vector.tensor_scalar_mul(f2_bf[:], f2_f[:], 0.5)
            w["f2_bf"] = f2_bf
            W.append(w)

    # =======================================================================
    # helper: layernorm of feature-major [128, n] slice (stats over
    # partitions via PE ones-matmuls; broadcast back via PE).
    # =======================================================================
    work = ctx.enter_context(tc.tile_pool(name="work", bufs=2))

    def ln_cols(x_sl, n, out_dt=F32):
        sq = work.tile([P, BC], F32, tag="sq", name="sq")[:, :n]
        nc.vector.tensor_mul(sq[:], x_sl, x_sl)
        ps_st = psum(p_sm, [1, 2 * n])
        nc.tensor.matmul(ps_st[:, 0:n], ones_col[:], x_sl)
        nc.tensor.matmul(ps_st[:, n:2 * n], ones_col[:], sq[:])
        mean = work.tile([1, BC], F32, tag="mmr", name="mmr")[:, :n]
        nc.vector.tensor_scalar(mean[:], ps_st[:, 0:n], 1.0 / E, None,
                                op0=OP.mult)
        vh = work.tile([1, BC], F32, tag="var", name="var")[:, :n]
        nc.vector.tensor_scalar(vh[:], ps_st[:, n:2 * n], 0.5 / E, EPS * 0.5,
                                op0=OP.mult, op1=OP.add)
        m2 = work.tile([1, BC], F32, tag="m2r", name="m2r")[:, :n]
        nc.vector.tensor_mul(m2[:], mean[:], mean[:])
        nc.vector.scalar_tensor_tensor(vh[:], m2[:], -0.5, vh[:],
                                       op0=OP.mult, op1=OP.add)
        srt = work.tile([1, BC], F32, tag="srt", name="srt")[:, :n]
        rsqrt_nr(vh[:], 1, n, "lc", out_sl=srt[:])
        ps_b = psum(p_sm, [P, 2 * n])
        nc.tensor.matmul(ps_b[:, 0:n], ones_row[:], mean[:])
        nc.tensor.matmul(ps_b[:, n:2 * n], ones_row[:], srt[:])
        xo = work.tile([P, BC], out_dt, tag="xo" + str(out_dt), name="xo")[:, :n]
        tmp = work.tile([P, BC], F32, tag="xt", name="xt")[:, :n]
        nc.vector.tensor_tensor(tmp[:], x_sl, ps_b[:, 0:n], op=OP.subtract)
        nc.vector.tensor_tensor(xo[:], tmp[:], ps_b[:, n:2 * n], op=OP.mult)
        return xo

    # =======================================================================
    # main loop over supergroups
    # =======================================================================
    big = ctx.enter_context(tc.tile_pool(name="big", bufs=2))
    graw = ctx.enter_context(tc.tile_pool(name="graw", bufs=2))
    st6p = ctx.enter_context(tc.tile_pool(name="st6p", bufs=2))

    nrm_tiles = {}
    NB8 = SG // 8

    def norm_sg(sg):
        """Normalize enc_out for one supergroup; stage-sweeped per 8 b."""
        b0 = sg * SG
        nrmN = big.tile([P, SG, NCH, E], BF16, tag="nrmN", name="nrmN")
        nrmT = big.tile([P, SG, T], BF16, tag="nrmT", name="nrmT")
        nrm_tiles[sg] = (nrmN, nrmT)
        for g in range(NB8):
            gb = g * 8
            encR = graw.tile([P, 8, NCH, E], BF16, tag="encR", name="encR")
            nc.gpsimd.dma_start(
                out=encR[:],
                in_=ins["enc_out"][b0 + gb:b0 + gb + 8].rearrange(
                    "b (c p) e -> p b c e", p=P))
            st6 = st6p.tile([P, 8, NCH, 6], F32, tag="st6", name="st6")
            for bl in range(8):
                for c in range(NCH):
                    nc.vector.bn_stats(st6[:, bl, c, :], encR[:, bl, c, :])
            # combine even/odd lane stats: mean=(m0+m1)/2,
            # vh=(var+eps)/2=(cv0+cv1)/(2*128) + ((m0-m1)/2)^2/2 + eps/2
            nst = 8 * NCH
            sview = st6[:].rearrange("p b c s -> p (b c) s")

            def sl(k):
                return tap(sview, k, [[6, nst]])

            mcol = st6p.tile([P, 8, NCH], F32, tag="mcol", name="mcol")
            rstd = st6p.tile([P, 8, NCH], F32, tag="rstd", name="rstd")
            dtmp = st6p.tile([P, nst], F32, tag="dtmp", name="dtmp")
            vtmp = st6p.tile([P, nst], F32, tag="vtmp", name="vtmp")
            mv = mcol[:].rearrange("p b c -> p (b c)")
            rv = rstd[:].rearrange("p b c -> p (b c)")
            nc.vector.tensor_tensor(mv, sl(1), sl(4), op=OP.add)
            nc.vector.tensor_scalar(mv, mv, 0.5, None, op0=OP.mult)
            nc.vector.tensor_tensor(dtmp[:], sl(1), sl(4), op=OP.subtract)
            nc.vector.tensor_mul(dtmp[:], dtmp[:], dtmp[:])
            nc.vector.tensor_tensor(vtmp[:], sl(2), sl(5), op=OP.add)
            nc.vector.tensor_scalar(vtmp[:], vtmp[:], 0.5 / E, EPS * 0.5,
                                    op0=OP.mult, op1=OP.add)
            nc.vector.scalar_tensor_tensor(vtmp[:], dtmp[:], 0.125, vtmp[:],
                                           op0=OP.mult, op1=OP.add)
            rsqrt_nr(vtmp[:], P, nst, "ns", out_sl=rv)
            for bl in range(8):
                b = gb + bl
                for c in range(NCH):
                    nc.vector.tensor_scalar(
                        nrmN[:, b, c, :], encR[:, bl, c, :],
                        mcol[:, bl, c:c + 1], rstd[:, bl, c:c + 1],
                        op0=OP.subtract, op1=OP.mult)
            # transpose the whole 8-b group via the DMA xbar in ONE
            # instruction: in [t_p, (b c e)] -> out[e, (b c), t_p], which is
            # exactly nrmT's [e, b, (c t_p)] layout. Frees PE (32 matmuls) and
            # DVE/ACT (8 PSUM->SBUF copies) per group at ~1.3us of Sync issue.
            nc.sync.dma_start_transpose(
                nrmT[:, gb:gb + 8, :].rearrange("p b (c q) -> p (b c) q",
                                                c=NCH),
                nrmN[:, gb:gb + 8, :, :].rearrange("p b c e -> p (b c e)"))

    norm_sg(0)
    for sg in range(NSG):
        if sg + 1 < NSG:
            norm_sg(sg + 1)
        nrmN, nrmT = nrm_tiles.pop(sg)
        if stage <= 2:
            continue

        # ---------------- decoder layers ----------------
        b0 = sg * SG
        sl_q = q_T[:, b0:b0 + SG]
        for l in range(L):
            w = W[l]
            # ---- q-side: ln3 -> Q -> U' [e, b, h]
            qx3 = ln_cols(sl_q, SG, out_dt=BF16)
            ps_Q = psum(p_sm, [16, H, SG])
            for h in range(H):
                nc.tensor.matmul(ps_Q[:, h, :],
                                 w["wq_e"][:, 16 * h:16 * h + 16], qx3[:],
                                 start=(h == 0), stop=False)
            # qb bias for all heads in one K=8 matmul against the indicator
            nc.tensor.matmul(ps_Q[:].rearrange("p h s -> p (h s)"),
                             w["qbT"][:], ind8[:].rearrange("p a b -> p (a b)"),
                             start=False, stop=True)
            Q_spl = work.tile([16, H, SG], BF16, tag="Qspl")
            nc.vector.tensor_copy(Q_spl[:].rearrange("p h s -> p (h s)"),
                                  ps_Q[:].rearrange("p h s -> p (h s)"))
            ps_U = psum(p_sm, [P, H, SG])
            for h in range(H):
                nc.tensor.matmul(ps_U[:, h, :], w["wk_spl"][:, h, :],
                                 Q_spl[:, h, :])
            # U' stored b-major [e, b, h] so quad weight slices are contiguous;
            # the copy reads ps_U [e, h, b] with a reordering AP.
            U_sb = work.tile([P, SG, H], BF16, tag="Usb")
            nc.scalar.activation(
                U_sb[:].rearrange("p b h -> p (b h)"),
                tap(ps_U[:], 0, [[1, SG], [SG, H]]),
                AF.Copy, scale=w["g1"][:])

            if stage <= 3:
                continue

            # ---- attention: 8 quads of 4 b, software-pipelined by stage so
            # each engine's FIFO queue never blocks on another engine's
            # in-flight work.
            NQ = SG // 4
            rnT_all = work.tile([P, SG, H], BF16, tag="rnT")
            qt = [dict() for _ in range(NQ)]

            def st_scores(q):
                ps_s = psum(p_sc, [P, T])
                qt[q]["ps_s"] = ps_s
                for m in range(4):
                    nc.tensor.matmul(
                        ps_s[32 * m:32 * m + 32, :],
                        U_sb[:, 4 * q:4 * q + 4, :], nrmT[:, 4 * q + m, :],
                        tile_position=(0, 32 * m))

            def st_exp(q):
                expw = work.tile([P, T], BF16, tag="expw")
                dsum = work.tile([P, 1], F32, tag="dsum")
                qt[q]["expw"] = expw
                qt[q]["dsum"] = dsum
                nc.scalar.activation(expw[:], qt[q]["ps_s"][:], AF.Exp,
                                     scale=float(D ** 0.5),
                                     accum_out=dsum[:])

            def st_wt(q):
                ps_w = psum(p_wt, [P, NCH, P])
                qt[q]["ps_w"] = ps_w
                expw = qt[q]["expw"]
                for c in range(NCH):
                    nc.tensor.matmul(ps_w[:, c, :],
                                     expw[:, c * P:(c + 1) * P], id_bf[:])

            def st_wc(q):
                weiT = work.tile([P, NCH, P], BF16, tag="weiT")
                qt[q]["weiT"] = weiT
                src = qt[q]["ps_w"][:].rearrange("p c q -> p (c q)")
                dst = weiT[:].rearrange("p c q -> p (c q)")
                if q % 2 == 0:
                    nc.scalar.copy(dst, src)
                else:
                    nc.vector.tensor_copy(dst, src)

            def st_val(q):
                ps_v = psum(p_sm, [P, E])
                qt[q]["ps_v"] = ps_v
                weiT = qt[q]["weiT"]
                for c in range(NCH):
                    for m in range(4):
                        nc.tensor.matmul(
                            ps_v[32 * m:32 * m + 32, :],
                            weiT[:, c, 32 * m:32 * m + 32],
                            nrmN[:, 4 * q + m, c, :],
                            tile_position=(0, 32 * m),
                            start=(c == 0), stop=(c == NCH - 1))

            def st_fix(q):
                ps_v = qt[q]["ps_v"]
                dinv = work.tile([P, 1], F32, tag="dinv")
                nc.vector.reciprocal(dinv[:], qt[q]["dsum"][:])
                rn_q = work.tile([P, E], BF16, tag="rnq")
                qt[q]["rn_q"] = rn_q
                nc.vector.tensor_scalar_mul(rn_q[:], ps_v[:], dinv[:])

            def st_rnt(q):
                ps_r = psum(p_tr, [P, P])
                qt[q]["ps_r"] = ps_r
                nc.tensor.matmul(ps_r[:], qt[q]["rn_q"][:], id_bf[:])

            def st_gat(q):
                nc.scalar.copy(
                    rnT_all[:, 4 * q:4 * q + 4, :],
                    tap(qt[q]["ps_r"][:], 0, [[40, 4], [1, 8]]))
                qt[q].clear()

            stages = [st_scores, st_exp, st_wt, st_wc, st_val, st_fix,
                      st_rnt, st_gat]
            for step in range(NQ + len(stages) - 1):
                for si in range(len(stages) - 1, -1, -1):
                    q = step - si
                    if 0 <= q < NQ:
                        stages[si](q)

            # ---- att @ wv, then proj back to residual
            ps_at = psum(p_sm, [16, H, SG])
            for h in range(H):
                nc.tensor.matmul(ps_at[:, h, :],
                                 w["wv_bf"][:, 16 * h:16 * h + 16],
                                 rnT_all[:, :, h])
            att_sb = work.tile([16, H, SG], BF16, tag="attsb")
            nc.scalar.copy(att_sb[:].rearrange("p h b -> p (h b)"),
                           ps_at[:].rearrange("p h b -> p (h b)"))
            ps_p = psum(p_sm, [P, SG])
            for h in range(H):
                nc.tensor.matmul(ps_p[:], w["pj_bf"][:, h, :], att_sb[:, h, :],
                                 start=(h == 0), stop=(h == H - 1))
            nc.vector.scalar_tensor_tensor(sl_q, ps_p[:], w["bias2"][:], sl_q,
                                           op0=OP.add, op1=OP.add)

            if stage <= 4:
                continue

            # ---- FFN (gelu-tanh evaluated on the full [P, 4*SG] batch;
            # f1 bias added via a K=4 indicator matmul, 0.5 folded into f2,
            # and gelu split as (f2/2)@g + (f2/2)@(g*tanh))
            qx4 = ln_cols(sl_q, SG, out_dt=BF16)
            ps_h = psum(p_sm, [P, 4, SG])
            ps_hv = ps_h[:].rearrange("p m s -> p (m s)")
            for m in range(4):
                nc.tensor.matmul(ps_h[:, m, :],
                                 w["f1_e"][:, m * E:(m + 1) * E], qx4[:],
                                 start=(m == 0), stop=False)
            nc.tensor.matmul(ps_hv, w["fb1eT"][:],
                             ind8[0:4, 0:4, :].rearrange("p a b -> p (a b)"),
                             start=False, stop=True)
            g_bf = work.tile([P, 4, SG], BF16, tag="gbf")
            nc.vector.tensor_copy(g_bf[:].rearrange("p m s -> p (m s)"), ps_hv)
            g2 = work.tile([P, 4 * SG], F32, tag="gx2")
            nc.scalar.activation(g2[:], ps_hv, AF.Square)
            nc.vector.tensor_scalar(g2[:], g2[:], 0.044715, 1.0,
                                    op0=OP.mult, op1=OP.add)
            nc.vector.tensor_tensor(g2[:], g2[:], ps_hv, op=OP.mult)
            th = work.tile([P, 4, SG], BF16, tag="gth")
            nc.scalar.activation(th[:].rearrange("p m s -> p (m s)"), g2[:],
                                 AF.Tanh, scale=0.7978845608028654)
            gt = work.tile([P, 4, SG], BF16, tag="hgb")
            nc.vector.tensor_mul(gt[:].rearrange("p m s -> p (m s)"),
                                 g_bf[:].rearrange("p m s -> p (m s)"),
                                 th[:].rearrange("p m s -> p (m s)"))
            ps_f = psum(p_sm, [P, SG])
            for k in range(4):
                nc.tensor.matmul(ps_f[:], w["f2_bf"][:, k, :], g_bf[:, k, :],
                                 start=(k == 0), stop=False)
                nc.tensor.matmul(ps_f[:], w["f2_bf"][:, k, :], gt[:, k, :],
                                 start=False, stop=(k == 3))
            nc.vector.scalar_tensor_tensor(sl_q, ps_f[:], w["fb2"][:], sl_q,
                                           op0=OP.add, op1=OP.add)

    if stage <= 5:
        dbg_out(q_T)
        return

    # ---------------- final layernorm + head ----------------
    # Output is produced batch-major ([b, 37] with b on partitions) so the
    # store DMA writes contiguous 148B rows instead of 9472 4-byte packets.
    hw_t = wts.tile([E, 37], F32, tag="hw")
    dma(out=hw_t[:], in_=ins["head_w"])
    gf = load_col("lnf_g", None, P, "gf")
    bf = load_col("lnf_b", None, P, "bf")
    hb_row = wts.tile([1, 37], F32, tag="hb")
    dma(out=hb_row[:], in_=ins["head_b"])
    hw_e = wts.tile([E, 37], BF16, tag="hwe")
    nc.vector.tensor_scalar_mul(hw_e[:], hw_t[:], gf[:])
    ps4 = psum(p_sm, [1, 37])
    nc.tensor.matmul(ps4[:], bf[:], hw_t[:])
    hbe_row = wts.tile([1, 37], F32, tag="hbe")
    nc.vector.tensor_add(hbe_row[:], ps4[:], hb_row[:])
    qxf = ln_cols(q_T[:], BC, out_dt=BF16)
    out_sb = work.tile([P, 2, 37], F32, tag="osb")
    for k in range(2):
        ps_o = psum(p_sc, [P, 37])
        nc.tensor.matmul(ps_o[:], qxf[:, k * P:(k + 1) * P], hw_e[:],
                         start=True, stop=False)
        nc.tensor.matmul(ps_o[:], ones_row[:], hbe_row[:],
                         start=False, stop=True)
        nc.vector.tensor_copy(out_sb[:, k, :], ps_o[:])
    dma(out=out_ap.rearrange("(k p) o -> p k o", p=P), in_=out_sb[:])


def input_specs_for(BC):
    full = [
        ("enc_out", [BC, T, E]), ("x1", [BC, 3, 37, 1]), ("x2", [BC, 7, 4, 1]),
        ("x3", [BC, 4]),
        ("c11_w", [8, 3, 3, 3]), ("c11_b", [8]), ("bn11_g", [8]), ("bn11_b", [8]),
        ("c12_w", [8, 8, 3, 3]), ("c12_b", [8]), ("bn12_g", [8]), ("bn12_b", [8]),
        ("fc1_w", [296, 64]), ("fc1_b", [64]),
        ("c21_w", [8, 7, 3, 3]), ("c21_b", [8]), ("bn21_g", [8]), ("bn21_b", [8]),
        ("fc2_w", [32, 16]), ("fc2_b", [16]),
        ("fc_w", [84, 128]), ("fc_b", [128]),
        ("wk", [L, E, E]), ("wq", [L, E, E]), ("wv", [L, E, E]),
        ("proj_w", [L, E, E]), ("proj_b", [L, E]),
        ("ln1_g", [L, E]), ("ln1_b", [L, E]), ("ln2_g", [L, E]), ("ln2_b", [L, E]),
        ("ln3_g", [L, E]), ("ln3_b", [L, E]), ("ln4_g", [L, E]), ("ln4_b", [L, E]),
        ("ff_w1", [L, E, 4 * E]), ("ff_b1", [L, 4 * E]),
        ("ff_w2", [L, 4 * E, E]), ("ff_b2", [L, E]),
        ("lnf_g", [E]), ("lnf_b", [E]), ("head_w", [E, 37]), ("head_b", [37]),
    ]
    return [(n, s, F32) for n, s in full]


def build_program(BC=256, stage=99):
    nc = bacc.Bacc("TRN2", target_bir_lowering=False, debug=False,
                   enable_asserts=True, num_devices=1)
    ins = {}
    for name, shape, dt_ in input_specs_for(BC):
        ins[name] = nc.dram_tensor(name, shape, dt_, kind="ExternalInput").ap()
    out_ap = nc.dram_tensor("out", [BC, 37], F32, kind="ExternalOutput").ap()
    with tile.TileContext(nc) as tc:
        with ExitStack() as ctx:
            decoder_body(ctx, tc, out_ap, ins, BC, stage=stage)
    nc.compile()
    return nc


_prog_cache = {}


def kernel(**inputs):
    BC = B_FULL // N_CORES
    if BC not in _prog_cache:
        _prog_cache[BC] = build_program(BC)
    nc = _prog_cache[BC]
    in_maps = []
    for c in range(N_CORES):
        m = {}
        for name, shape, _ in input_specs_for(BC):
            arr = np.ascontiguousarray(np.asarray(inputs[name], dtype=np.float32))
            if name in SHARDED:
                arr = arr[c * BC:(c + 1) * BC]
            m[name] = np.ascontiguousarray(arr)
        in_maps.append(m)
    res = run_bass_kernel_spmd(nc, in_maps, core_ids=list(range(N_CORES)))
    return np.concatenate([r["out"] for r in res.results], axis=0)



# revision 34
# speedup vs baseline: 1.2040x; 1.2040x over previous
"""Trainium2 Bass kernel for nn_Decoder (dense_transformer) — v2.

Key restructuring vs v1 baseline (1532us):
  - Attention matmuls are PE-efficient: scores stream nrm_T (N=512) against a
    32-col stationary U' slice, 4 batch elements packed into one PSUM bank via
    tile_position col-tiling; values stream nrm_nat (N=129) against 32-col
    transposed-softmax weights. No more 128-col weight reloads per (b,chunk).
  - The softmax denominator comes free from an appended ones-column in the
    values rhs (col 128), so rn = ps[:,0:128] * recip(ps[:,128]).
  - All transposes are regular matmuls with a bf16 identity rhs (keeps the PE
    HAM-warm at 2.4GHz, unlike transpose-mode).
  - enc_out is cast fp32->bf16 in the DMA (SWDGE), halving DVE stats/apply
    cost; bn_stats runs per-b (4 chunks in one op) and the mean/var
    aggregation is done with a handful of [128,64] strided ops per group.
  - Layer weights are loaded + folded ONCE (not per supergroup).

Sharding: pure data parallel, batch 2048 -> 8 cores x 256.
"""

import math
from contextlib import ExitStack

import numpy as np

import concourse.bass as bass
import concourse.tile as tile
from concourse import bacc, mybir
from concourse.bass_utils import run_bass_kernel_spmd
from concourse.masks import make_identity

F32 = mybir.dt.float32
BF16 = mybir.dt.bfloat16
I32 = mybir.dt.int32
AF = mybir.ActivationFunctionType
OP = mybir.AluOpType

RSQRT_MAGIC_H = 0x5EF759DF  # quake magic 0x5f3759df shifted for vh = v/2 seed

P = 128
T = 512
E = 128
H = 8
D = 16
L = 3
NCH = T // P            # 4 t-chunks
BN_S = 1.0 / math.sqrt(1.0 + 1e-5)
EPS = 1e-5
N_CORES = 8
B_FULL = 2048
SG = 32                 # supergroup batch size (residual-stream width)
# softmax denominators come from the exp activation's accum_out, so the
# values rhs is just the E normalized columns (no appended ones column).

SHARDED = ("enc_out", "x1", "x2", "x3")


def _ap(t, offset, pattern):
    return bass.AP(tensor=t.tensor, offset=offset, ap=[list(p) for p in pattern])


def tap(ap, extra_off, free_pattern):
    """Sub-AP of a tile AP: keep partition dim, replace free dims."""
    return bass.AP(tensor=ap.tensor, offset=ap.offset + extra_off,
                   ap=[list(ap.ap[0])] + [list(p) for p in free_pattern])


def statenet(ctx, tc, ins, q_T, BC, p_a, p_b):
    """Conv/FC front-end producing q0 [E, BC] into q_T. (v1 logic verbatim.)"""
    nc = tc.nc
    dma = nc.sync.dma_start
    NB = (BC + P - 1) // P

    def psum(pool, shape, dt_=F32):
        return pool.tile(shape, dt_, tag=pool.name, name=pool.name + "_t")

    id_f32 = ctx._id_f32

    with tc.tile_pool(name="snet", bufs=1) as sn:
        x1T = sn.tile([111, BC], F32)
        x2T = sn.tile([28, BC], F32)
        cat64 = sn.tile([64, BC], F32)
        cat16 = sn.tile([16, BC], F32)
        x3c = sn.tile([4, BC], F32)
        x3T = x3c[0:4, :]
        x1_f = ins["x1"].rearrange("b c h w -> b (c h w)")
        x2_f = ins["x2"].rearrange("b c h w -> b (c h w)")
        for i in range(NB):
            n = min(P, BC - i * P)
            for (srcx, dstT, w) in ((x1_f, x1T[:], 111), (x2_f, x2T[:], 28),
                                    (ins["x3"], x3T, 4)):
                xin = sn.tile([P, w], F32, tag="xin")
                dma(out=xin[:n, :], in_=srcx[i * P:i * P + n, :])
                pst = psum(p_a, [w, P])
                nc.tensor.transpose(pst[:, :n], xin[:n, :], id_f32[:n, :n])
                nc.scalar.copy(dstT[:, i * P:i * P + n], pst[:, :n])

        def conv_w(dram_ap, O_, C_, gname, bname, cbname):
            KK = C_ * 3
            ws = sn.tile([O_, KK], F32, tag="ws" + gname)
            dma(out=ws[:], in_=_ap(dram_ap, 1, [[C_ * 9, O_], [9, C_], [3, 3]]))
            g = sn.tile([O_, 1], F32, tag="g" + gname)
            dma(out=g[:], in_=ins[gname])
            gp = sn.tile([O_, 1], F32, tag="gp" + gname)
            nc.scalar.mul(gp[:], g[:], BN_S)
            cb = sn.tile([O_, 1], F32, tag="cb" + gname)
            dma(out=cb[:], in_=ins[cbname])
            bb = sn.tile([O_, 1], F32, tag="bb" + gname)
            dma(out=bb[:], in_=ins[bname])
            beff = sn.tile([O_, 1], F32, tag="be" + gname)
            nc.vector.tensor_mul(beff[:], cb[:], gp[:])
            nc.vector.tensor_add(beff[:], beff[:], bb[:])
            wsc = sn.tile([O_, KK], F32, tag="wsc" + gname)
            nc.vector.tensor_scalar_mul(wsc[:], ws[:], gp[:])
            pswt = psum(p_a, [KK, O_])
            nc.tensor.transpose(pswt[:], wsc[:], id_f32[:O_, :O_])
            wT = sn.tile([KK, O_], F32, tag="wT" + gname)
            nc.scalar.copy(wT[:], pswt[:])
            return wT, beff

        w1T, b1e = conv_w(ins["c11_w"], 8, 3, "bn11_g", "bn11_b", "c11_b")
        w2T, b2e = conv_w(ins["c12_w"], 8, 8, "bn12_g", "bn12_b", "c12_b")
        w3T, b3e = conv_w(ins["c21_w"], 8, 7, "bn21_g", "bn21_b", "c21_b")

        def im2col(srcT, C_, W_):
            rhs = sn.tile([24, 37, BC], F32, tag="im", name="imt")[:C_ * 3, :W_, :]
            nc.vector.memset(rhs[:], 0.0)
            for c in range(C_):
                for kh in range(3):
                    lo = max(0, 1 - kh)
                    hi = min(W_, W_ + 1 - kh)
                    n = hi - lo
                    s0 = c * W_ + lo + kh - 1
                    k_ = c * 3 + kh
                    dma(out=rhs[k_:k_ + 1, lo:hi, :], in_=srcT[s0:s0 + n, :])
            return rhs

        def conv_apply(rhs, wT, beff, O_, W_):
            y = sn.tile([8, 37, BC], F32, tag="yt", name="ytt")[:O_, :W_, :]
            step = max(1, 512 // BC)
            for i0 in range(0, W_, step):
                n = min(step, W_ - i0)
                psc = psum(p_b, [O_, step, BC])
                nc.tensor.matmul(psc[:, :n, :], wT[:], rhs[:, i0:i0 + n, :])
                nc.scalar.activation(y[:, i0:i0 + n, :], psc[:, :n, :],
                                     AF.Relu, bias=beff[:])
            return y

        r9 = im2col(x1T, 3, 37)
        y1 = conv_apply(r9, w1T, b1e, 8, 37)
        r24 = sn.tile([24, 37, BC], F32, tag="im", name="imt")
        nc.vector.memset(r24[:], 0.0)
        for c in range(8):
            for kh in range(3):
                lo = max(0, 1 - kh)
                hi = min(37, 37 + 1 - kh)
                n = hi - lo
                k_ = c * 3 + kh
                dma(out=r24[k_:k_ + 1, lo:hi, :],
                    in_=y1[c:c + 1, lo + kh - 1:lo + kh - 1 + n, :])
        y2 = conv_apply(r24, w2T, b2e, 8, 37)

        r21 = im2col(x2T, 7, 4)
        y2b = conv_apply(r21, w3T, b3e, 8, 4)

        y2r = []
        for k, (ilo, ihi) in enumerate(((0, 16), (16, 32), (32, 37))):
            ni = ihi - ilo
            t_ = sn.tile([ni * 8, BC], F32, tag=f"y2r{k}")
            for o in range(8):
                dma(out=t_[o * ni:(o + 1) * ni, :], in_=y2[o:o + 1, ilo:ihi, :])
            y2r.append(t_)
        y2br = sn.tile([32, BC], F32)
        for o in range(8):
            dma(out=y2br[o * 4:(o + 1) * 4, :], in_=y2b[o:o + 1, :, :])

        ps_h1 = psum(p_b, [64, BC])
        for k, (ilo, ihi) in enumerate(((0, 16), (16, 32), (32, 37))):
            ni = ihi - ilo
            fw = sn.tile([ni * 8, 64], F32, tag=f"fw{k}")
            dma(out=fw[:], in_=_ap(ins["fc1_w"], ilo * 64,
                                   [[37 * 64, 8], [64, ni], [1, 64]]))
            nc.tensor.matmul(ps_h1[:], fw[:], y2r[k][:],
                             start=(k == 0), stop=(k == 2))
        fb1 = sn.tile([64, 1], F32)
        dma(out=fb1[:], in_=ins["fc1_b"])
        h1 = cat64[0:64, :]
        nc.scalar.activation(h1, ps_h1[:], AF.Relu, bias=fb1[:])

        fw2 = sn.tile([32, 16], F32)
        dma(out=fw2[:], in_=ins["fc2_w"])
        ps_h2 = psum(p_b, [16, BC])
        nc.tensor.matmul(ps_h2[:], fw2[:], y2br[:])
        fb2 = sn.tile([16, 1], F32)
        dma(out=fb2[:], in_=ins["fc2_b"])
        h2 = cat16[0:16, :]
        nc.scalar.activation(h2, ps_h2[:], AF.Relu, bias=fb2[:])

        fcw64 = sn.tile([64, E], F32)
        dma(out=fcw64[:], in_=ins["fc_w"][0:64, :])
        fcw16 = sn.tile([16, E], F32)
        dma(out=fcw16[:], in_=ins["fc_w"][64:80, :])
        fcw3 = sn.tile([4, E], F32)
        dma(out=fcw3[:], in_=ins["fc_w"][80:84, :])
        ps_q0 = psum(p_b, [P, BC])
        nc.tensor.matmul(ps_q0[:], fcw64[:], cat64[:], start=True, stop=False)
        nc.tensor.matmul(ps_q0[:], fcw16[:], cat16[:], start=False, stop=False)
        nc.tensor.matmul(ps_q0[:], fcw3[:], x3c[:], start=False, stop=True)
        fcb = sn.tile([P, 1], F32)
        dma(out=fcb[:], in_=ins["fc_b"])
        nc.scalar.activation(q_T[:], ps_q0[:], AF.Relu, bias=fcb[:])


def decoder_body(ctx: ExitStack, tc: tile.TileContext, out_ap: bass.AP,
                 ins: dict, BC: int, stage: int = 99):
    nc = tc.nc
    dma = nc.sync.dma_start
    NSG = BC // SG

    def dbg_out(tag_ap):
        nc.sync.dma_start(out=out_ap.rearrange("b o -> o b"),
                          in_=tag_ap[0:37, 0:BC])

    # ---------------- pools ----------------
    const = ctx.enter_context(tc.tile_pool(name="const", bufs=1))
    wts = ctx.enter_context(tc.tile_pool(name="wts", bufs=1))
    perm = ctx.enter_context(tc.tile_pool(name="perm", bufs=1))
    # PSUM: 8 banks; each pool = bufs x max-2KB tile
    p_sc = ctx.enter_context(tc.tile_pool(name="p_sc", bufs=2, space="PSUM"))
    p_tr = ctx.enter_context(tc.tile_pool(name="p_tr", bufs=2, space="PSUM"))
    p_wt = ctx.enter_context(tc.tile_pool(name="p_wt", bufs=2, space="PSUM"))
    p_sm = ctx.enter_context(tc.tile_pool(name="p_sm", bufs=2, space="PSUM"))

    def psum(pool, shape, dt_=F32):
        return pool.tile(shape, dt_, tag=pool.name, name=pool.name + "_t")

    id_f32 = const.tile([P, P], F32)
    id_bf = const.tile([P, P], BF16)
    make_identity(nc, id_f32[:])
    make_identity(nc, id_bf[:])
    ctx._id_f32 = id_f32
    ones_col = const.tile([P, 1], F32)
    nc.vector.memset(ones_col[:], 1.0)
    ones_row = const.tile([1, P], F32)
    nc.vector.memset(ones_row[:], 1.0)
    # Newton-rsqrt constants (all-DVE rstd; keeps Sqrt off ScalarE so the
    # activation table never swaps away from exp_and_others). Single tuned
    # Newton step y0*(A - B*vh*y0^2): max rel err 8.8e-4.
    magic_t = const.tile([P, BC], I32)
    nc.vector.memset(magic_t[:], RSQRT_MAGIC_H)
    ca_t = const.tile([P, BC], F32)
    nc.vector.memset(ca_t[:], 1.50133365)
    # ind8[h, h'*32+j] = (h == h'): K=8 indicator used to add per-head/-chunk
    # biases with a single accumulating matmul (rows 0:4, cols 0:128 double as
    # the K=4 FFN-bias indicator).
    ind8 = const.tile([8, 8, SG], BF16)
    ones_row_bf = const.tile([1, SG], BF16)
    nc.vector.memset(ones_row_bf[:], 1.0)
    nc.vector.memset(ind8[:], 0.0)
    for hh in range(8):
        nc.sync.dma_start(out=ind8[hh:hh + 1, hh, :], in_=ones_row_bf[:])

    def rsqrt_nr(vh_sl, p_, n_, tagp, out_sl=None):
        """rstd = 1/sqrt(2*vh) via bit-trick seed + 2 Newton iters (DVE only).

        vh_sl: [p_, n_] f32 AP holding (var + eps) / 2. Writes into out_sl
        if given (returns it), else into a scratch tile."""
        ti = work.tile([p_, n_], I32, tag=tagp + "ti", name=tagp + "ti")
        y = work.tile([p_, n_], F32, tag=tagp + "y", name=tagp + "y")
        t = work.tile([p_, n_], F32, tag=tagp + "t", name=tagp + "t")
        nc.vector.tensor_scalar(ti[:], vh_sl.bitcast(I32), 1, None,
                                op0=OP.logical_shift_right)
        nc.vector.tensor_tensor(y[:].bitcast(I32), magic_t[0:p_, 0:n_],
                                ti[:], op=OP.subtract)
        nc.vector.tensor_mul(t[:], y[:], y[:])
        nc.vector.tensor_tensor(t[:], vh_sl, t[:], op=OP.mult)
        nc.vector.scalar_tensor_tensor(t[:], t[:], -1.00091486,
                                       ca_t[0:p_, 0:n_],
                                       op0=OP.mult, op1=OP.add)
        dst = y[:] if out_sl is None else out_sl
        nc.vector.tensor_tensor(dst, y[:], t[:], op=OP.mult)
        return y[:] if out_sl is None else out_sl

    q_T = perm.tile([P, BC], F32)            # persistent residual [E, b]

    # =======================================================================
    # StateNet (scoped; its SBUF is reclaimed before the big pools open)
    # =======================================================================
    statenet(ctx, tc, ins, q_T, BC, p_sm, p_sc)
    if stage <= 1:
        dbg_out(q_T)
        return

    # =======================================================================
    # Phase 0: load + fold all layer weights once
    # =======================================================================
    def load_col(name, l, n, tg):
        t_ = wts.tile([n, 1], F32, tag=tg)
        src = ins[name]
        dma(out=t_[:], in_=src[l] if l is not None else src)
        return t_

    W = []  # per-layer dict of folded weights (raw loads live in a scope)
    with tc.tile_pool(name="wraw", bufs=1) as wr:
        for l in range(L):
            w = {}
            wq_t = wr.tile([E, E], F32, tag="wq")
            dma(out=wq_t[:], in_=ins["wq"][l])
            wk_t = wr.tile([E, E], F32, tag="wk", name="wk_t")
            dma(out=wk_t[:], in_=ins["wk"][l])
            pj_t = wr.tile([E, E], F32, tag="pj", name="pj_t")
            dma(out=pj_t[:], in_=ins["proj_w"][l])
            wv_t = wr.tile([E, E], F32, tag="wv", name="wv_t")
            dma(out=wv_t[:], in_=ins["wv"][l])
            g1 = load_col("ln1_g", l, P, f"g1{l}")
            g2 = wr.tile([P, 1], F32, tag="g2")
            dma(out=g2[:], in_=ins["ln2_g"][l])
            b2 = wr.tile([P, 1], F32, tag="b2")
            dma(out=b2[:], in_=ins["ln2_b"][l])
            g3 = wr.tile([P, 1], F32, tag="g3")
            dma(out=g3[:], in_=ins["ln3_g"][l])
            b3 = load_col("ln3_b", l, P, f"b3{l}")
            g4 = wr.tile([P, 1], F32, tag="g4")
            dma(out=g4[:], in_=ins["ln4_g"][l])
            b4 = load_col("ln4_b", l, P, f"b4{l}")
            pjb = wr.tile([P, 1], F32, tag="pjb")
            dma(out=pjb[:], in_=ins["proj_b"][l])
            w["g1"], w["b3"], w["b4"] = g1, b3, b4
            w["fb2"] = load_col("ff_b2", l, P, f"fb2{l}")

            wq_e = wts.tile([E, E], BF16, tag=f"wqe{l}")
            nc.vector.tensor_scalar_mul(wq_e[:], wq_t[:], g3[:])
            w["wq_e"] = wq_e
            qb_ps = psum(p_sm, [16, H])
            for h in range(H):
                nc.tensor.matmul(qb_ps[:, h:h + 1],
                                 wq_t[:, 16 * h:16 * h + 16], b3[:])
            qb_spl = wr.tile([16, H], F32, tag="qb", name="qb_spl")
            nc.scalar.copy(qb_spl[:], qb_ps[:])
            ps_qbT = psum(p_sm, [H, 16])
            nc.tensor.matmul(ps_qbT[:], qb_spl[:], id_f32[0:16, 0:16])
            qbT = wts.tile([H, 16], BF16, tag=f"qbT{l}")
            nc.scalar.copy(qbT[:], ps_qbT[:])
            w["qbT"] = qbT

            wk_spl = wts.tile([16, H, E], BF16, tag=f"wks{l}")
            for hh in range(2):
                ps_kT = psum(p_sm, [16, 4, E])
                for h4 in range(4):
                    h = hh * 4 + h4
                    nc.tensor.transpose(ps_kT[:, h4, :],
                                        wk_t[:, 16 * h:16 * h + 16], id_f32[:])
                nc.scalar.copy(wk_spl[:, 4 * hh:4 * hh + 4, :], ps_kT[:])
            w["wk_spl"] = wk_spl

            wv_e = wr.tile([E, E], F32, tag="wve", name="wv_e")
            nc.vector.tensor_scalar_mul(wv_e[:], wv_t[:], g2[:])
            wv_bf = wts.tile([E, E], BF16, tag=f"wvbf{l}")
            nc.vector.tensor_copy(wv_bf[:], wv_e[:])
            w["wv_bf"] = wv_bf
            ps2 = psum(p_sm, [P, 1])
            nc.tensor.matmul(ps2[:], wv_e[:], b2[:])
            c2 = wr.tile([P, 1], F32, tag="c2", name="c2")
            nc.scalar.copy(c2[:], ps2[:])
            ps2b = psum(p_sm, [P, 1])
            nc.tensor.matmul(ps2b[:], pj_t[:], c2[:])
            bias2 = wts.tile([P, 1], F32, tag=f"bias2{l}")
            nc.vector.tensor_add(bias2[:], ps2b[:], pjb[:])
            w["bias2"] = bias2

            pj_bf = wts.tile([16, H, E], BF16, tag=f"pjs{l}")
            pj_f = wr.tile([16, H, E], F32, tag="pjf", name="pj_f")
            dma(out=pj_f[:], in_=_ap(ins["proj_w"], l * E * E,
                                     [[E, 16], [16 * E, H], [1, E]]))
            nc.vector.tensor_copy(pj_bf[:], pj_f[:])
            w["pj_bf"] = pj_bf

            f1_t = wr.tile([E, 4 * E], F32, tag="f1", name="f1_t")
            dma(out=f1_t[:], in_=ins["ff_w1"][l])
            f1_e = wts.tile([E, 4 * E], BF16, tag=f"f1e{l}")
            f1_ef = wr.tile([E, 4 * E], F32, tag="f1ef", name="f1_ef")
            nc.vector.tensor_scalar_mul(f1_ef[:], f1_t[:], g4[:])
            nc.vector.tensor_copy(f1_e[:], f1_ef[:])
            w["f1_e"] = f1_e
            ps3 = psum(p_sm, [P, 4])
            for m in range(4):
                nc.tensor.matmul(ps3[:, m:m + 1], f1_ef[:, m * E:(m + 1) * E],
                                 b4[:])
            fb1_ = wr.tile([P, 4], F32, tag="fb1", name="fb1_")
            dma(out=fb1_[:], in_=ins["ff_b1"][l].rearrange("(c p) -> p c", p=P))
            fb1e = wr.tile([P, 4], F32, tag="fb1e", name="fb1e")
            nc.vector.tensor_add(fb1e[:], ps3[:], fb1_[:])
            ps_bT = psum(p_sm, [4, P])
            nc.tensor.matmul(ps_bT[:], fb1e[:], id_f32[:])
            fb1eT = wts.tile([4, P], BF16, tag=f"fb1eT{l}")
            nc.scalar.copy(fb1eT[:], ps_bT[:])
            w["fb1eT"] = fb1eT

            f2_f = wr.tile([P, 4, E], F32, tag="f2f", name="f2_f")
            dma(out=f2_f[:],
                in_=ins["ff_w2"][l].rearrange("(c p) e -> p c e", p=P))
            # halved so gelu = (f2/2)@g + (f2/2)@(g*tanh) needs no +1/×0.5 ops
            f2_bf = wts.tile([P, 4, E], BF16, tag=f"f2{l}")
            nc.vector.tensor_scalar_mul(f2_bf[:], f2_f[:], 0.5)
            w["f2_bf"] = f2_bf
            W.append(w)

    # =======================================================================
    # helper: layernorm of feature-major [128, n] slice (stats over
    # partitions via PE ones-matmuls; broadcast back via PE).
    # =======================================================================
    work = ctx.enter_context(tc.tile_pool(name="work", bufs=2))

    def ln_cols(x_sl, n, out_dt=F32):
        sq = work.tile([P, BC], F32, tag="sq", name="sq")[:, :n]
        nc.vector.tensor_mul(sq[:], x_sl, x_sl)
        ps_st = psum(p_sm, [1, 2 * n])
        nc.tensor.matmul(ps_st[:, 0:n], ones_col[:], x_sl)
        nc.tensor.matmul(ps_st[:, n:2 * n], ones_col[:], sq[:])
        mean = work.tile([1, BC], F32, tag="mmr", name="mmr")[:, :n]
        nc.vector.tensor_scalar(mean[:], ps_st[:, 0:n], 1.0 / E, None,
                                op0=OP.mult)
        vh = work.tile([1, BC], F32, tag="var", name="var")[:, :n]
        nc.vector.tensor_scalar(vh[:], ps_st[:, n:2 * n], 0.5 / E, EPS * 0.5,
                                op0=OP.mult, op1=OP.add)
        m2 = work.tile([1, BC], F32, tag="m2r", name="m2r")[:, :n]
        nc.vector.tensor_mul(m2[:], mean[:], mean[:])
        nc.vector.scalar_tensor_tensor(vh[:], m2[:], -0.5, vh[:],
                                       op0=OP.mult, op1=OP.add)
        srt = work.tile([1, BC], F32, tag="srt", name="srt")[:, :n]
        rsqrt_nr(vh[:], 1, n, "lc", out_sl=srt[:])
        ps_b = psum(p_sm, [P, 2 * n])
        nc.tensor.matmul(ps_b[:, 0:n], ones_row[:], mean[:])
        nc.tensor.matmul(ps_b[:, n:2 * n], ones_row[:], srt[:])
        xo = work.tile([P, BC], out_dt, tag="xo" + str(out_dt), name="xo")[:, :n]
        tmp = work.tile([P, BC], F32, tag="xt", name="xt")[:, :n]
        nc.vector.tensor_tensor(tmp[:], x_sl, ps_b[:, 0:n], op=OP.subtract)
        nc.vector.tensor_tensor(xo[:], tmp[:], ps_b[:, n:2 * n], op=OP.mult)
        return xo

    # =======================================================================
    # main loop over supergroups
    # =======================================================================
    big = ctx.enter_context(tc.tile_pool(name="big", bufs=2))
    graw = ctx.enter_context(tc.tile_pool(name="graw", bufs=2))
    st6p = ctx.enter_context(tc.tile_pool(name="st6p", bufs=2))

    nrm_tiles = {}
    NB8 = SG // 8

    def norm_sg(sg):
        """Normalize enc_out for one supergroup; stage-sweeped per 8 b."""
        b0 = sg * SG
        nrmN = big.tile([P, SG, NCH, E], BF16, tag="nrmN", name="nrmN")
        nrmT = big.tile([P, SG, T], BF16, tag="nrmT", name="nrmT")
        nrm_tiles[sg] = (nrmN, nrmT)
        for g in range(NB8):
            gb = g * 8
            encR = graw.tile([P, 8, NCH, E], BF16, tag="encR", name="encR")
            nc.gpsimd.dma_start(
                out=encR[:],
                in_=ins["enc_out"][b0 + gb:b0 + gb + 8].rearrange(
                    "b (c p) e -> p b c e", p=P))
            st6 = st6p.tile([P, 8, NCH, 6], F32, tag="st6", name="st6")
            for bl in range(8):
                for c in range(NCH):
                    nc.vector.bn_stats(st6[:, bl, c, :], encR[:, bl, c, :])
            # combine even/odd lane stats: mean=(m0+m1)/2,
            # vh=(var+eps)/2=(cv0+cv1)/(2*128) + ((m0-m1)/2)^2/2 + eps/2
            nst = 8 * NCH
            sview = st6[:].rearrange("p b c s -> p (b c) s")

            def sl(k):
                return tap(sview, k, [[6, nst]])

            mcol = st6p.tile([P, 8, NCH], F32, tag="mcol", name="mcol")
            rstd = st6p.tile([P, 8, NCH], F32, tag="rstd", name="rstd")
            dtmp = st6p.tile([P, nst], F32, tag="dtmp", name="dtmp")
            vtmp = st6p.tile([P, nst], F32, tag="vtmp", name="vtmp")
            mv = mcol[:].rearrange("p b c -> p (b c)")
            rv = rstd[:].rearrange("p b c -> p (b c)")
            nc.vector.tensor_tensor(mv, sl(1), sl(4), op=OP.add)
            nc.vector.tensor_scalar(mv, mv, 0.5, None, op0=OP.mult)
            nc.vector.tensor_tensor(dtmp[:], sl(1), sl(4), op=OP.subtract)
            nc.vector.tensor_mul(dtmp[:], dtmp[:], dtmp[:])
            nc.vector.tensor_tensor(vtmp[:], sl(2), sl(5), op=OP.add)
            nc.vector.tensor_scalar(vtmp[:], vtmp[:], 0.5 / E, EPS * 0.5,
                                    op0=OP.mult, op1=OP.add)
            nc.vector.scalar_tensor_tensor(vtmp[:], dtmp[:], 0.125, vtmp[:],
                                           op0=OP.mult, op1=OP.add)
            rsqrt_nr(vtmp[:], P, nst, "ns", out_sl=rv)
            # negmr = -mean*rstd lets ScalarE normalize one chunk per b via
            # its free affine (out = Identity(x*rstd + (-mean*rstd))),
            # unloading the Vector engine (the busiest).
            negmr = st6p.tile([P, 8, NCH], F32, tag="negmr", name="negmr")
            nc.vector.scalar_tensor_tensor(
                negmr[:].rearrange("p b c -> p (b c)"), mv, -1.0, rv,
                op0=OP.mult, op1=OP.mult)
            for bl in range(8):
                b = gb + bl
                for c in range(3):
                    nc.vector.tensor_scalar(
                        nrmN[:, b, c, :], encR[:, bl, c, :],
                        mcol[:, bl, c:c + 1], rstd[:, bl, c:c + 1],
                        op0=OP.subtract, op1=OP.mult)
                nc.scalar.activation(
                    nrmN[:, b, 3, :], encR[:, bl, 3, :], AF.Identity,
                    bias=negmr[:, bl, 3:4], scale=rstd[:, bl, 3:4])
            for bl in range(8):
                b = gb + bl
                ps_t = psum(p_tr, [P, NCH, P])
                for c in range(NCH):
                    nc.tensor.matmul(ps_t[:, c, :], nrmN[:, b, c, :],
                                     id_bf[:])
                if bl % 2 == 0:
                    nc.scalar.copy(nrmT[:, b, :],
                                   ps_t[:].rearrange("p c q -> p (c q)"))
                else:
                    nc.vector.tensor_copy(
                        nrmT[:, b, :], ps_t[:].rearrange("p c q -> p (c q)"))

    norm_sg(0)
    for sg in range(NSG):
        if sg + 1 < NSG:
            norm_sg(sg + 1)
        nrmN, nrmT = nrm_tiles.pop(sg)
        if stage <= 2:
            continue

        # ---------------- decoder layers ----------------
        b0 = sg * SG
        sl_q = q_T[:, b0:b0 + SG]
        for l in range(L):
            w = W[l]
            # ---- q-side: ln3 -> Q -> U' [e, b, h]
            qx3 = ln_cols(sl_q, SG, out_dt=BF16)
            ps_Q = psum(p_sm, [16, H, SG])
            for h in range(H):
                nc.tensor.matmul(ps_Q[:, h, :],
                                 w["wq_e"][:, 16 * h:16 * h + 16], qx3[:],
                                 start=(h == 0), stop=False)
            # qb bias for all heads in one K=8 matmul against the indicator
            nc.tensor.matmul(ps_Q[:].rearrange("p h s -> p (h s)"),
                             w["qbT"][:], ind8[:].rearrange("p a b -> p (a b)"),
                             start=False, stop=True)
            Q_spl = work.tile([16, H, SG], BF16, tag="Qspl")
            nc.vector.tensor_copy(Q_spl[:].rearrange("p h s -> p (h s)"),
                                  ps_Q[:].rearrange("p h s -> p (h s)"))
            ps_U = psum(p_sm, [P, H, SG])
            for h in range(H):
                nc.tensor.matmul(ps_U[:, h, :], w["wk_spl"][:, h, :],
                                 Q_spl[:, h, :])
            # U' stored b-major [e, b, h] so quad weight slices are contiguous;
            # the copy reads ps_U [e, h, b] with a reordering AP.
            U_sb = work.tile([P, SG, H], BF16, tag="Usb")
            nc.scalar.activation(
                U_sb[:].rearrange("p b h -> p (b h)"),
                tap(ps_U[:], 0, [[1, SG], [SG, H]]),
                AF.Copy, scale=w["g1"][:])

            if stage <= 3:
                continue

            # ---- attention: 8 quads of 4 b, software-pipelined by stage so
            # each engine's FIFO queue never blocks on another engine's
            # in-flight work.
            NQ = SG // 4
            rnT_all = work.tile([P, SG, H], BF16, tag="rnT")
            qt = [dict() for _ in range(NQ)]

            def st_scores(q):
                ps_s = psum(p_sc, [P, T])
                qt[q]["ps_s"] = ps_s
                for m in range(4):
                    nc.tensor.matmul(
                        ps_s[32 * m:32 * m + 32, :],
                        U_sb[:, 4 * q:4 * q + 4, :], nrmT[:, 4 * q + m, :],
                        tile_position=(0, 32 * m))

            def st_exp(q):
                expw = work.tile([P, T], BF16, tag="expw")
                dsum = work.tile([P, 1], F32, tag="dsum")
                qt[q]["expw"] = expw
                qt[q]["dsum"] = dsum
                nc.scalar.activation(expw[:], qt[q]["ps_s"][:], AF.Exp,
                                     scale=float(D ** 0.5),
                                     accum_out=dsum[:])

            def st_wt(q):
                ps_w = psum(p_wt, [P, NCH, P])
                qt[q]["ps_w"] = ps_w
                expw = qt[q]["expw"]
                for c in range(NCH):
                    nc.tensor.matmul(ps_w[:, c, :],
                                     expw[:, c * P:(c + 1) * P], id_bf[:])

            def st_wc(q):
                weiT = work.tile([P, NCH, P], BF16, tag="weiT")
                qt[q]["weiT"] = weiT
                src = qt[q]["ps_w"][:].rearrange("p c q -> p (c q)")
                dst = weiT[:].rearrange("p c q -> p (c q)")
                if q % 2 == 0:
                    nc.scalar.copy(dst, src)
                else:
                    nc.vector.tensor_copy(dst, src)

            def st_val(q):
                ps_v = psum(p_sm, [P, E])
                qt[q]["ps_v"] = ps_v
                weiT = qt[q]["weiT"]
                for c in range(NCH):
                    for m in range(4):
                        nc.tensor.matmul(
                            ps_v[32 * m:32 * m + 32, :],
                            weiT[:, c, 32 * m:32 * m + 32],
                            nrmN[:, 4 * q + m, c, :],
                            tile_position=(0, 32 * m),
                            start=(c == 0), stop=(c == NCH - 1))

            def st_fix(q):
                ps_v = qt[q]["ps_v"]
                dinv = work.tile([P, 1], F32, tag="dinv")
                nc.vector.reciprocal(dinv[:], qt[q]["dsum"][:])
                rn_q = work.tile([P, E], BF16, tag="rnq")
                qt[q]["rn_q"] = rn_q
                nc.scalar.activation(rn_q[:], ps_v[:], AF.Identity,
                                     scale=dinv[:])

            def st_rnt(q):
                ps_r = psum(p_tr, [P, P])
                qt[q]["ps_r"] = ps_r
                nc.tensor.matmul(ps_r[:], qt[q]["rn_q"][:], id_bf[:])

            def st_gat(q):
                nc.scalar.copy(
                    rnT_all[:, 4 * q:4 * q + 4, :],
                    tap(qt[q]["ps_r"][:], 0, [[40, 4], [1, 8]]))
                qt[q].clear()

            stages = [st_scores, st_exp, st_wt, st_wc, st_val, st_fix,
                      st_rnt, st_gat]
            for step in range(NQ + len(stages) - 1):
                for si in range(len(stages) - 1, -1, -1):
                    q = step - si
                    if 0 <= q < NQ:
                        stages[si](q)

            # ---- att @ wv, then proj back to residual
            ps_at = psum(p_sm, [16, H, SG])
            for h in range(H):
                nc.tensor.matmul(ps_at[:, h, :],
                                 w["wv_bf"][:, 16 * h:16 * h + 16],
                                 rnT_all[:, :, h])
            att_sb = work.tile([16, H, SG], BF16, tag="attsb")
            nc.scalar.copy(att_sb[:].rearrange("p h b -> p (h b)"),
                           ps_at[:].rearrange("p h b -> p (h b)"))
            ps_p = psum(p_sm, [P, SG])
            for h in range(H):
                nc.tensor.matmul(ps_p[:], w["pj_bf"][:, h, :], att_sb[:, h, :],
                                 start=(h == 0), stop=(h == H - 1))
            nc.vector.scalar_tensor_tensor(sl_q, ps_p[:], w["bias2"][:], sl_q,
                                           op0=OP.add, op1=OP.add)

            if stage <= 4:
                continue

            # ---- FFN (gelu-tanh evaluated on the full [P, 4*SG] batch;
            # f1 bias added via a K=4 indicator matmul, 0.5 folded into f2,
            # and gelu split as (f2/2)@g + (f2/2)@(g*tanh))
            qx4 = ln_cols(sl_q, SG, out_dt=BF16)
            ps_h = psum(p_sm, [P, 4, SG])
            ps_hv = ps_h[:].rearrange("p m s -> p (m s)")
            for m in range(4):
                nc.tensor.matmul(ps_h[:, m, :],
                                 w["f1_e"][:, m * E:(m + 1) * E], qx4[:],
                                 start=(m == 0), stop=False)
            nc.tensor.matmul(ps_hv, w["fb1eT"][:],
                             ind8[0:4, 0:4, :].rearrange("p a b -> p (a b)"),
                             start=False, stop=True)
            g_bf = work.tile([P, 4, SG], BF16, tag="gbf")
            nc.vector.tensor_copy(g_bf[:].rearrange("p m s -> p (m s)"), ps_hv)
            g2 = work.tile([P, 4 * SG], F32, tag="gx2")
            nc.scalar.activation(g2[:], ps_hv, AF.Square)
            nc.vector.tensor_scalar(g2[:], g2[:], 0.044715, 1.0,
                                    op0=OP.mult, op1=OP.add)
            nc.vector.tensor_tensor(g2[:], g2[:], ps_hv, op=OP.mult)
            th = work.tile([P, 4, SG], BF16, tag="gth")
            nc.scalar.activation(th[:].rearrange("p m s -> p (m s)"), g2[:],
                                 AF.Tanh, scale=0.7978845608028654)
            gt = work.tile([P, 4, SG], BF16, tag="hgb")
            nc.vector.tensor_mul(gt[:].rearrange("p m s -> p (m s)"),
                                 g_bf[:].rearrange("p m s -> p (m s)"),
                                 th[:].rearrange("p m s -> p (m s)"))
            ps_f = psum(p_sm, [P, SG])
            for k in range(4):
                nc.tensor.matmul(ps_f[:], w["f2_bf"][:, k, :], g_bf[:, k, :],
                                 start=(k == 0), stop=False)
                nc.tensor.matmul(ps_f[:], w["f2_bf"][:, k, :], gt[:, k, :],
                                 start=False, stop=(k == 3))
            nc.vector.scalar_tensor_tensor(sl_q, ps_f[:], w["fb2"][:], sl_q,
                                           op0=OP.add, op1=OP.add)

    if stage <= 5:
        dbg_out(q_T)
        return

    # ---------------- final layernorm + head ----------------
    # Output is produced batch-major ([b, 37] with b on partitions) so the
    # store DMA writes contiguous 148B rows instead of 9472 4-byte packets.
    hw_t = wts.tile([E, 37], F32, tag="hw")
    dma(out=hw_t[:], in_=ins["head_w"])
    gf = load_col("lnf_g", None, P, "gf")
    bf = load_col("lnf_b", None, P, "bf")
    hb_row = wts.tile([1, 37], F32, tag="hb")
    dma(out=hb_row[:], in_=ins["head_b"])
    hw_e = wts.tile([E, 37], BF16, tag="hwe")
    nc.vector.tensor_scalar_mul(hw_e[:], hw_t[:], gf[:])
    ps4 = psum(p_sm, [1, 37])
    nc.tensor.matmul(ps4[:], bf[:], hw_t[:])
    hbe_row = wts.tile([1, 37], F32, tag="hbe")
    nc.vector.tensor_add(hbe_row[:], ps4[:], hb_row[:])
    qxf = ln_cols(q_T[:], BC, out_dt=BF16)
    out_sb = work.tile([P, 2, 37], F32, tag="osb")
    for k in range(2):
        ps_o = psum(p_sc, [P, 37])
        nc.tensor.matmul(ps_o[:], qxf[:, k * P:(k + 1) * P], hw_e[:],
                         start=True, stop=False)
        nc.tensor.matmul(ps_o[:], ones_row[:], hbe_row[:],
                         start=False, stop=True)
        nc.vector.tensor_copy(out_sb[:, k, :], ps_o[:])
    dma(out=out_ap.rearrange("(k p) o -> p k o", p=P), in_=out_sb[:])


def input_specs_for(BC):
    full = [
        ("enc_out", [BC, T, E]), ("x1", [BC, 3, 37, 1]), ("x2", [BC, 7, 4, 1]),
        ("x3", [BC, 4]),
        ("c11_w", [8, 3, 3, 3]), ("c11_b", [8]), ("bn11_g", [8]), ("bn11_b", [8]),
        ("c12_w", [8, 8, 3, 3]), ("c12_b", [8]), ("bn12_g", [8]), ("bn12_b", [8]),
        ("fc1_w", [296, 64]), ("fc1_b", [64]),
        ("c21_w", [8, 7, 3, 3]), ("c21_b", [8]), ("bn21_g", [8]), ("bn21_b", [8]),
        ("fc2_w", [32, 16]), ("fc2_b", [16]),
        ("fc_w", [84, 128]), ("fc_b", [128]),
        ("wk", [L, E, E]), ("wq", [L, E, E]), ("wv", [L, E, E]),
        ("proj_w", [L, E, E]), ("proj_b", [L, E]),
        ("ln1_g", [L, E]), ("ln1_b", [L, E]), ("ln2_g", [L, E]), ("ln2_b", [L, E]),
        ("ln3_g", [L, E]), ("ln3_b", [L, E]), ("ln4_g", [L, E]), ("ln4_b", [L, E]),
        ("ff_w1", [L, E, 4 * E]), ("ff_b1", [L, 4 * E]),
        ("ff_w2", [L, 4 * E, E]), ("ff_b2", [L, E]),
        ("lnf_g", [E]), ("lnf_b", [E]), ("head_w", [E, 37]), ("head_b", [37]),
    ]
    return [(n, s, F32) for n, s in full]


def build_program(BC=256, stage=99):
    nc = bacc.Bacc("TRN2", target_bir_lowering=False, debug=False,
                   enable_asserts=True, num_devices=1)
    ins = {}
    for name, shape, dt_ in input_specs_for(BC):
        ins[name] = nc.dram_tensor(name, shape, dt_, kind="ExternalInput").ap()
    out_ap = nc.dram_tensor("out", [BC, 37], F32, kind="ExternalOutput").ap()
    with tile.TileContext(nc) as tc:
        with ExitStack() as ctx:
            decoder_body(ctx, tc, out_ap, ins, BC, stage=stage)
    nc.compile()
    return nc


_prog_cache = {}


def kernel(**inputs):
    BC = B_FULL // N_CORES
    if BC not in _prog_cache:
        _prog_cache[BC] = build_program(BC)
    nc = _prog_cache[BC]
    in_maps = []
    for c in range(N_CORES):
        m = {}
        for name, shape, _ in input_specs_for(BC):
            arr = np.ascontiguousarray(np.asarray(inputs[name], dtype=np.float32))
            if name in SHARDED:
                arr = arr[c * BC:(c + 1) * BC]
            m[name] = np.ascontiguousarray(arr)
        in_maps.append(m)
    res = run_bass_kernel_spmd(nc, in_maps, core_ids=list(range(N_CORES)))
    return np.concatenate([r["out"] for r in res.results], axis=0)



# revision 50
# speedup vs baseline: 1.2601x; 1.0467x over previous
"""Trainium2 Bass kernel for nn_Decoder (dense_transformer) — v3.

v3 changes vs v2 (2038us -> ~1390us measured, rel err 5.2e-3):
  - No ScalarE Sqrt anywhere: rstd comes from an all-DVE Newton rsqrt
    (bit-trick seed on (var+eps)/2, one tuned iteration). This keeps the
    activation table pinned on exp_and_others (exp/tanh/square/identity),
    eliminating ~122 ACT_TABLE_LOADs (156us of ScalarE).
  - Head emits the output batch-major via two qxf-stationary matmuls +
    a rank-1 bias matmul, so the store DMA writes 148B rows instead of
    9473 4-byte packets (was a 123us serial tail).
  - FFN gelu evaluated once per (sg,l) on [128, 4*SG] with the f1 bias
    added by a K=4 indicator matmul; gelu split as (f2/2)@g + (f2/2)@(g*t)
    with 0.5 folded into the f2 weights.
  - Q-path biases via one K=8 indicator matmul; Q_spl is a single cast.
    wq_e/qx3 in bf16 (fp32 matmuls run LOW/HIGH double-pass on PE).
  - Softmax denominators from the exp activation's accum_out (the ones
    column in the values rhs is gone); st_fix normalize runs on ScalarE.
  - One of four normalize-apply chunks per b runs on ScalarE via its
    free affine (scale=rstd, bias=-mean*rstd).

Key restructuring vs v1 baseline (1532us):
  - Attention matmuls are PE-efficient: scores stream nrm_T (N=512) against a
    32-col stationary U' slice, 4 batch elements packed into one PSUM bank via
    tile_position col-tiling; values stream nrm_nat (N=129) against 32-col
    transposed-softmax weights. No more 128-col weight reloads per (b,chunk).
  - The softmax denominator comes free from an appended ones-column in the
    values rhs (col 128), so rn = ps[:,0:128] * recip(ps[:,128]).
  - All transposes are regular matmuls with a bf16 identity rhs (keeps the PE
    HAM-warm at 2.4GHz, unlike transpose-mode).
  - enc_out is cast fp32->bf16 in the DMA (SWDGE), halving DVE stats/apply
    cost; bn_stats runs per-b (4 chunks in one op) and the mean/var
    aggregation is done with a handful of [128,64] strided ops per group.
  - Layer weights are loaded + folded ONCE (not per supergroup).

Sharding: pure data parallel, batch 2048 -> 8 cores x 256.
"""

import math
from contextlib import ExitStack

import numpy as np

import concourse.bass as bass
import concourse.tile as tile
from concourse import bacc, mybir
from concourse.bass_utils import run_bass_kernel_spmd
from concourse.masks import make_identity

F32 = mybir.dt.float32
BF16 = mybir.dt.bfloat16
I32 = mybir.dt.int32
AF = mybir.ActivationFunctionType
OP = mybir.AluOpType

RSQRT_MAGIC_H = 0x5EF759DF  # quake magic 0x5f3759df shifted for vh = v/2 seed

P = 128
T = 512
E = 128
H = 8
D = 16
L = 3
NCH = T // P            # 4 t-chunks
BN_S = 1.0 / math.sqrt(1.0 + 1e-5)
EPS = 1e-5
N_CORES = 8
B_FULL = 2048
SG = 32                 # supergroup batch size (residual-stream width)
# softmax denominators come from the exp activation's accum_out, so the
# values rhs is just the E normalized columns (no appended ones column).

SHARDED = ("enc_out", "x1", "x2", "x3")


def _ap(t, offset, pattern):
    return bass.AP(tensor=t.tensor, offset=offset, ap=[list(p) for p in pattern])


def tap(ap, extra_off, free_pattern):
    """Sub-AP of a tile AP: keep partition dim, replace free dims."""
    return bass.AP(tensor=ap.tensor, offset=ap.offset + extra_off,
                   ap=[list(ap.ap[0])] + [list(p) for p in free_pattern])


def statenet(ctx, tc, ins, q_T, BC, p_a, p_b):
    """Conv/FC front-end producing q0 [E, BC] into q_T. (v1 logic verbatim.)"""
    nc = tc.nc
    dma = nc.sync.dma_start
    NB = (BC + P - 1) // P

    def psum(pool, shape, dt_=F32):
        return pool.tile(shape, dt_, tag=pool.name, name=pool.name + "_t")

    id_f32 = ctx._id_f32
    id_bf = ctx._id_bf

    # conv/fc tensors are bf16 so the PE runs single-pass matmuls
    # (fp32 operands lower to LOW/HIGH double matmuls).
    with tc.tile_pool(name="snet", bufs=1) as sn:
        x1T = sn.tile([111, BC], BF16)
        x2T = sn.tile([28, BC], BF16)
        cat64 = sn.tile([64, BC], BF16)
        cat16 = sn.tile([16, BC], BF16)
        x3c = sn.tile([4, BC], BF16)
        x3T = x3c[0:4, :]
        x1_f = ins["x1"].rearrange("b c h w -> b (c h w)")
        x2_f = ins["x2"].rearrange("b c h w -> b (c h w)")
        for i in range(NB):
            n = min(P, BC - i * P)
            for (srcx, dstT, w) in ((x1_f, x1T[:], 111), (x2_f, x2T[:], 28),
                                    (ins["x3"], x3T, 4)):
                xin = sn.tile([P, w], F32, tag="xin")
                dma(out=xin[:n, :], in_=srcx[i * P:i * P + n, :])
                pst = psum(p_a, [w, P])
                nc.tensor.transpose(pst[:, :n], xin[:n, :], id_f32[:n, :n])
                nc.scalar.copy(dstT[:, i * P:i * P + n], pst[:, :n])

        def conv_w(dram_ap, O_, C_, gname, bname, cbname):
            KK = C_ * 3
            ws = sn.tile([O_, KK], F32, tag="ws" + gname)
            dma(out=ws[:], in_=_ap(dram_ap, 1, [[C_ * 9, O_], [9, C_], [3, 3]]))
            g = sn.tile([O_, 1], F32, tag="g" + gname)
            dma(out=g[:], in_=ins[gname])
            gp = sn.tile([O_, 1], F32, tag="gp" + gname)
            nc.scalar.mul(gp[:], g[:], BN_S)
            cb = sn.tile([O_, 1], F32, tag="cb" + gname)
            dma(out=cb[:], in_=ins[cbname])
            bb = sn.tile([O_, 1], F32, tag="bb" + gname)
            dma(out=bb[:], in_=ins[bname])
            beff = sn.tile([O_, 1], F32, tag="be" + gname)
            nc.vector.tensor_mul(beff[:], cb[:], gp[:])
            nc.vector.tensor_add(beff[:], beff[:], bb[:])
            wsc = sn.tile([O_, KK], BF16, tag="wsc" + gname)
            nc.vector.tensor_scalar_mul(wsc[:], ws[:], gp[:])
            pswt = psum(p_a, [KK, O_])
            nc.tensor.matmul(pswt[:], wsc[:], id_bf[:O_, :O_])
            wT = sn.tile([KK, O_], BF16, tag="wT" + gname)
            nc.scalar.copy(wT[:], pswt[:])
            return wT, beff

        w1T, b1e = conv_w(ins["c11_w"], 8, 3, "bn11_g", "bn11_b", "c11_b")
        w2T, b2e = conv_w(ins["c12_w"], 8, 8, "bn12_g", "bn12_b", "c12_b")
        w3T, b3e = conv_w(ins["c21_w"], 8, 7, "bn21_g", "bn21_b", "c21_b")

        def im2col(srcT, C_, W_):
            rhs = sn.tile([24, 37, BC], BF16, tag="im", name="imt")[:C_ * 3, :W_, :]
            nc.vector.memset(rhs[:], 0.0)
            for c in range(C_):
                for kh in range(3):
                    lo = max(0, 1 - kh)
                    hi = min(W_, W_ + 1 - kh)
                    n = hi - lo
                    s0 = c * W_ + lo + kh - 1
                    k_ = c * 3 + kh
                    dma(out=rhs[k_:k_ + 1, lo:hi, :], in_=srcT[s0:s0 + n, :])
            return rhs

        def conv_apply(rhs, wT, beff, O_, W_):
            y = sn.tile([8, 37, BC], BF16, tag="yt", name="ytt")[:O_, :W_, :]
            step = max(1, 512 // BC)
            for i0 in range(0, W_, step):
                n = min(step, W_ - i0)
                psc = psum(p_b, [O_, step, BC])
                nc.tensor.matmul(psc[:, :n, :], wT[:], rhs[:, i0:i0 + n, :])
                nc.scalar.activation(y[:, i0:i0 + n, :], psc[:, :n, :],
                                     AF.Relu, bias=beff[:])
            return y

        r9 = im2col(x1T, 3, 37)
        y1 = conv_apply(r9, w1T, b1e, 8, 37)
        r24 = sn.tile([24, 37, BC], BF16, tag="im", name="imt")
        nc.vector.memset(r24[:], 0.0)
        for c in range(8):
            for kh in range(3):
                lo = max(0, 1 - kh)
                hi = min(37, 37 + 1 - kh)
                n = hi - lo
                k_ = c * 3 + kh
                dma(out=r24[k_:k_ + 1, lo:hi, :],
                    in_=y1[c:c + 1, lo + kh - 1:lo + kh - 1 + n, :])
        y2 = conv_apply(r24, w2T, b2e, 8, 37)

        r21 = im2col(x2T, 7, 4)
        y2b = conv_apply(r21, w3T, b3e, 8, 4)

        y2r = []
        for k, (ilo, ihi) in enumerate(((0, 16), (16, 32), (32, 37))):
            ni = ihi - ilo
            t_ = sn.tile([ni * 8, BC], BF16, tag=f"y2r{k}")
            for o in range(8):
                dma(out=t_[o * ni:(o + 1) * ni, :], in_=y2[o:o + 1, ilo:ihi, :])
            y2r.append(t_)
        y2br = sn.tile([32, BC], BF16)
        for o in range(8):
            dma(out=y2br[o * 4:(o + 1) * 4, :], in_=y2b[o:o + 1, :, :])

        ps_h1 = psum(p_b, [64, BC])
        for k, (ilo, ihi) in enumerate(((0, 16), (16, 32), (32, 37))):
            ni = ihi - ilo
            fw = sn.tile([ni * 8, 64], BF16, tag=f"fw{k}")
            nc.gpsimd.dma_start(out=fw[:], in_=_ap(ins["fc1_w"], ilo * 64,
                                [[37 * 64, 8], [64, ni], [1, 64]]))
            nc.tensor.matmul(ps_h1[:], fw[:], y2r[k][:],
                             start=(k == 0), stop=(k == 2))
        fb1 = sn.tile([64, 1], F32)
        dma(out=fb1[:], in_=ins["fc1_b"])
        h1 = cat64[0:64, :]
        nc.scalar.activation(h1, ps_h1[:], AF.Relu, bias=fb1[:])

        fw2 = sn.tile([32, 16], BF16)
        nc.gpsimd.dma_start(out=fw2[:], in_=ins["fc2_w"])
        ps_h2 = psum(p_b, [16, BC])
        nc.tensor.matmul(ps_h2[:], fw2[:], y2br[:])
        fb2 = sn.tile([16, 1], F32)
        dma(out=fb2[:], in_=ins["fc2_b"])
        h2 = cat16[0:16, :]
        nc.scalar.activation(h2, ps_h2[:], AF.Relu, bias=fb2[:])

        fcw64 = sn.tile([64, E], BF16)
        nc.gpsimd.dma_start(out=fcw64[:], in_=ins["fc_w"][0:64, :])
        fcw16 = sn.tile([16, E], BF16)
        nc.gpsimd.dma_start(out=fcw16[:], in_=ins["fc_w"][64:80, :])
        fcw3 = sn.tile([4, E], BF16)
        nc.gpsimd.dma_start(out=fcw3[:], in_=ins["fc_w"][80:84, :])
        ps_q0 = psum(p_b, [P, BC])
        nc.tensor.matmul(ps_q0[:], fcw64[:], cat64[:], start=True, stop=False)
        nc.tensor.matmul(ps_q0[:], fcw16[:], cat16[:], start=False, stop=False)
        nc.tensor.matmul(ps_q0[:], fcw3[:], x3c[:], start=False, stop=True)
        fcb = sn.tile([P, 1], F32)
        dma(out=fcb[:], in_=ins["fc_b"])
        nc.scalar.activation(q_T[:], ps_q0[:], AF.Relu, bias=fcb[:])


def decoder_body(ctx: ExitStack, tc: tile.TileContext, out_ap: bass.AP,
                 ins: dict, BC: int, stage: int = 99):
    nc = tc.nc
    dma = nc.sync.dma_start
    NSG = BC // SG

    def dbg_out(tag_ap):
        nc.sync.dma_start(out=out_ap.rearrange("b o -> o b"),
                          in_=tag_ap[0:37, 0:BC])

    # ---------------- pools ----------------
    const = ctx.enter_context(tc.tile_pool(name="const", bufs=1))
    wts = ctx.enter_context(tc.tile_pool(name="wts", bufs=1))
    perm = ctx.enter_context(tc.tile_pool(name="perm", bufs=1))
    # PSUM: 8 banks; each pool = bufs x max-2KB tile
    p_sc = ctx.enter_context(tc.tile_pool(name="p_sc", bufs=2, space="PSUM"))
    p_tr = ctx.enter_context(tc.tile_pool(name="p_tr", bufs=2, space="PSUM"))
    p_wt = ctx.enter_context(tc.tile_pool(name="p_wt", bufs=2, space="PSUM"))
    p_sm = ctx.enter_context(tc.tile_pool(name="p_sm", bufs=2, space="PSUM"))
    # softmax denominators live 4 pipeline steps (st_exp -> st_fix); a 2-slot
    # tag would stall st_exp(q) on st_fix(q-2), collapsing the attention
    # pipeline depth. Tiny tiles, so give them a deep dedicated pool.
    dsp = ctx.enter_context(tc.tile_pool(name="dsp", bufs=8))

    def psum(pool, shape, dt_=F32):
        return pool.tile(shape, dt_, tag=pool.name, name=pool.name + "_t")

    id_f32 = const.tile([P, P], F32)
    id_bf = const.tile([P, P], BF16)
    make_identity(nc, id_f32[:])
    make_identity(nc, id_bf[:])
    ctx._id_f32 = id_f32
    ctx._id_bf = id_bf
    ones_col = const.tile([P, 1], F32)
    nc.vector.memset(ones_col[:], 1.0)
    ones_row = const.tile([1, P], F32)
    nc.vector.memset(ones_row[:], 1.0)
    # Newton-rsqrt constants (all-DVE rstd; keeps Sqrt off ScalarE so the
    # activation table never swaps away from exp_and_others). Single tuned
    # Newton step y0*(A - B*vh*y0^2): max rel err 8.8e-4.
    magic_t = const.tile([P, BC], I32)
    nc.vector.memset(magic_t[:], RSQRT_MAGIC_H)
    ca_t = const.tile([P, BC], F32)
    nc.vector.memset(ca_t[:], 1.50133365)
    # ind8[h, h'*32+j] = (h == h'): K=8 indicator used to add per-head/-chunk
    # biases with a single accumulating matmul (rows 0:4, cols 0:128 double as
    # the K=4 FFN-bias indicator).
    ind8 = const.tile([8, 8, SG], BF16)
    ones_row_bf = const.tile([1, SG], BF16)
    nc.vector.memset(ones_row_bf[:], 1.0)
    nc.vector.memset(ind8[:], 0.0)
    for hh in range(8):
        nc.sync.dma_start(out=ind8[hh:hh + 1, hh, :], in_=ones_row_bf[:])

    def rsqrt_nr(vh_sl, p_, n_, tagp, out_sl=None):
        """rstd = 1/sqrt(2*vh) via bit-trick seed + 2 Newton iters (DVE only).

        vh_sl: [p_, n_] f32 AP holding (var + eps) / 2. Writes into out_sl
        if given (returns it), else into a scratch tile."""
        ti = work.tile([p_, n_], I32, tag=tagp + "ti", name=tagp + "ti")
        y = work.tile([p_, n_], F32, tag=tagp + "y", name=tagp + "y")
        t = work.tile([p_, n_], F32, tag=tagp + "t", name=tagp + "t")
        nc.vector.tensor_scalar(ti[:], vh_sl.bitcast(I32), 1, None,
                                op0=OP.logical_shift_right)
        nc.vector.tensor_tensor(y[:].bitcast(I32), magic_t[0:p_, 0:n_],
                                ti[:], op=OP.subtract)
        nc.vector.tensor_mul(t[:], y[:], y[:])
        nc.vector.tensor_tensor(t[:], vh_sl, t[:], op=OP.mult)
        nc.vector.scalar_tensor_tensor(t[:], t[:], -1.00091486,
                                       ca_t[0:p_, 0:n_],
                                       op0=OP.mult, op1=OP.add)
        dst = y[:] if out_sl is None else out_sl
        nc.vector.tensor_tensor(dst, y[:], t[:], op=OP.mult)
        return y[:] if out_sl is None else out_sl

    q_T = perm.tile([P, BC], F32)            # persistent residual [E, b]

    # =======================================================================
    # StateNet (scoped; its SBUF is reclaimed before the big pools open)
    # =======================================================================
    statenet(ctx, tc, ins, q_T, BC, p_sm, p_sc)
    if stage <= 1:
        dbg_out(q_T)
        return

    # =======================================================================
    # Phase 0: load + fold all layer weights once
    # =======================================================================
    def load_col(name, l, n, tg):
        t_ = wts.tile([n, 1], F32, tag=tg)
        src = ins[name]
        dma(out=t_[:], in_=src[l] if l is not None else src)
        return t_

    W = []  # per-layer dict of folded weights (raw loads live in a scope)
    with tc.tile_pool(name="wraw", bufs=1) as wr:
        for l in range(L):
            w = {}
            wq_t = wr.tile([E, E], F32, tag="wq")
            dma(out=wq_t[:], in_=ins["wq"][l])
            wk_t = wr.tile([E, E], F32, tag="wk", name="wk_t")
            dma(out=wk_t[:], in_=ins["wk"][l])
            pj_t = wr.tile([E, E], F32, tag="pj", name="pj_t")
            dma(out=pj_t[:], in_=ins["proj_w"][l])
            wv_t = wr.tile([E, E], F32, tag="wv", name="wv_t")
            dma(out=wv_t[:], in_=ins["wv"][l])
            g1 = load_col("ln1_g", l, P, f"g1{l}")
            g2 = wr.tile([P, 1], F32, tag="g2")
            dma(out=g2[:], in_=ins["ln2_g"][l])
            b2 = wr.tile([P, 1], F32, tag="b2")
            dma(out=b2[:], in_=ins["ln2_b"][l])
            g3 = wr.tile([P, 1], F32, tag="g3")
            dma(out=g3[:], in_=ins["ln3_g"][l])
            b3 = load_col("ln3_b", l, P, f"b3{l}")
            g4 = wr.tile([P, 1], F32, tag="g4")
            dma(out=g4[:], in_=ins["ln4_g"][l])
            b4 = load_col("ln4_b", l, P, f"b4{l}")
            pjb = wr.tile([P, 1], F32, tag="pjb")
            dma(out=pjb[:], in_=ins["proj_b"][l])
            w["g1"], w["b3"], w["b4"] = g1, b3, b4
            w["fb2"] = load_col("ff_b2", l, P, f"fb2{l}")

            wq_e = wts.tile([E, E], BF16, tag=f"wqe{l}")
            nc.vector.tensor_scalar_mul(wq_e[:], wq_t[:], g3[:])
            w["wq_e"] = wq_e
            qb_ps = psum(p_sm, [16, H])
            for h in range(H):
                nc.tensor.matmul(qb_ps[:, h:h + 1],
                                 wq_t[:, 16 * h:16 * h + 16], b3[:])
            qb_spl = wr.tile([16, H], F32, tag="qb", name="qb_spl")
            nc.scalar.copy(qb_spl[:], qb_ps[:])
            ps_qbT = psum(p_sm, [H, 16])
            nc.tensor.matmul(ps_qbT[:], qb_spl[:], id_f32[0:16, 0:16])
            qbT = wts.tile([H, 16], BF16, tag=f"qbT{l}")
            nc.scalar.copy(qbT[:], ps_qbT[:])
            w["qbT"] = qbT

            wk_spl = wts.tile([16, H, E], BF16, tag=f"wks{l}")
            for hh in range(2):
                ps_kT = psum(p_sm, [16, 4, E])
                for h4 in range(4):
                    h = hh * 4 + h4
                    nc.tensor.transpose(ps_kT[:, h4, :],
                                        wk_t[:, 16 * h:16 * h + 16], id_f32[:])
                nc.scalar.copy(wk_spl[:, 4 * hh:4 * hh + 4, :], ps_kT[:])
            w["wk_spl"] = wk_spl

            wv_e = wr.tile([E, E], F32, tag="wve", name="wv_e")
            nc.vector.tensor_scalar_mul(wv_e[:], wv_t[:], g2[:])
            wv_bf = wts.tile([E, E], BF16, tag=f"wvbf{l}")
            nc.vector.tensor_copy(wv_bf[:], wv_e[:])
            w["wv_bf"] = wv_bf
            ps2 = psum(p_sm, [P, 1])
            nc.tensor.matmul(ps2[:], wv_e[:], b2[:])
            c2 = wr.tile([P, 1], F32, tag="c2", name="c2")
            nc.scalar.copy(c2[:], ps2[:])
            ps2b = psum(p_sm, [P, 1])
            nc.tensor.matmul(ps2b[:], pj_t[:], c2[:])
            bias2 = wts.tile([P, 1], F32, tag=f"bias2{l}")
            nc.vector.tensor_add(bias2[:], ps2b[:], pjb[:])
            w["bias2"] = bias2

            pj_bf = wts.tile([16, H, E], BF16, tag=f"pjs{l}")
            pj_f = wr.tile([16, H, E], F32, tag="pjf", name="pj_f")
            dma(out=pj_f[:], in_=_ap(ins["proj_w"], l * E * E,
                                     [[E, 16], [16 * E, H], [1, E]]))
            nc.vector.tensor_copy(pj_bf[:], pj_f[:])
            w["pj_bf"] = pj_bf

            f1_t = wr.tile([E, 4 * E], F32, tag="f1", name="f1_t")
            dma(out=f1_t[:], in_=ins["ff_w1"][l])
            f1_e = wts.tile([E, 4 * E], BF16, tag=f"f1e{l}")
            f1_ef = wr.tile([E, 4 * E], F32, tag="f1ef", name="f1_ef")
            nc.vector.tensor_scalar_mul(f1_ef[:], f1_t[:], g4[:])
            nc.vector.tensor_copy(f1_e[:], f1_ef[:])
            w["f1_e"] = f1_e
            ps3 = psum(p_sm, [P, 4])
            for m in range(4):
                nc.tensor.matmul(ps3[:, m:m + 1], f1_ef[:, m * E:(m + 1) * E],
                                 b4[:])
            fb1_ = wr.tile([P, 4], F32, tag="fb1", name="fb1_")
            dma(out=fb1_[:], in_=ins["ff_b1"][l].rearrange("(c p) -> p c", p=P))
            fb1e = wr.tile([P, 4], F32, tag="fb1e", name="fb1e")
            nc.vector.tensor_add(fb1e[:], ps3[:], fb1_[:])
            ps_bT = psum(p_sm, [4, P])
            nc.tensor.matmul(ps_bT[:], fb1e[:], id_f32[:])
            fb1eT = wts.tile([4, P], BF16, tag=f"fb1eT{l}")
            nc.scalar.copy(fb1eT[:], ps_bT[:])
            w["fb1eT"] = fb1eT

            f2_f = wr.tile([P, 4, E], F32, tag="f2f", name="f2_f")
            dma(out=f2_f[:],
                in_=ins["ff_w2"][l].rearrange("(c p) e -> p c e", p=P))
            # halved so gelu = (f2/2)@g + (f2/2)@(g*tanh) needs no +1/×0.5 ops
            f2_bf = wts.tile([P, 4, E], BF16, tag=f"f2{l}")
            nc.vector.tensor_scalar_mul(f2_bf[:], f2_f[:], 0.5)
            w["f2_bf"] = f2_bf
            W.append(w)

    # =======================================================================
    # helper: layernorm of feature-major [128, n] slice (stats over
    # partitions via PE ones-matmuls; broadcast back via PE).
    # =======================================================================
    work = ctx.enter_context(tc.tile_pool(name="work", bufs=2))

    def ln_cols(x_sl, n, out_dt=F32):
        sq = work.tile([P, BC], F32, tag="sq", name="sq")[:, :n]
        nc.vector.tensor_mul(sq[:], x_sl, x_sl)
        ps_st = psum(p_sm, [1, 2 * n])
        nc.tensor.matmul(ps_st[:, 0:n], ones_col[:], x_sl)
        nc.tensor.matmul(ps_st[:, n:2 * n], ones_col[:], sq[:])
        mean = work.tile([1, BC], F32, tag="mmr", name="mmr")[:, :n]
        nc.vector.tensor_scalar(mean[:], ps_st[:, 0:n], 1.0 / E, None,
                                op0=OP.mult)
        vh = work.tile([1, BC], F32, tag="var", name="var")[:, :n]
        nc.vector.tensor_scalar(vh[:], ps_st[:, n:2 * n], 0.5 / E, EPS * 0.5,
                                op0=OP.mult, op1=OP.add)
        m2 = work.tile([1, BC], F32, tag="m2r", name="m2r")[:, :n]
        nc.vector.tensor_mul(m2[:], mean[:], mean[:])
        nc.vector.scalar_tensor_tensor(vh[:], m2[:], -0.5, vh[:],
                                       op0=OP.mult, op1=OP.add)
        srt = work.tile([1, BC], F32, tag="srt", name="srt")[:, :n]
        rsqrt_nr(vh[:], 1, n, "lc", out_sl=srt[:])
        ps_b = psum(p_sm, [P, 2 * n])
        nc.tensor.matmul(ps_b[:, 0:n], ones_row[:], mean[:])
        nc.tensor.matmul(ps_b[:, n:2 * n], ones_row[:], srt[:])
        xo = work.tile([P, BC], out_dt, tag="xo" + str(out_dt), name="xo")[:, :n]
        tmp = work.tile([P, BC], F32, tag="xt", name="xt")[:, :n]
        nc.vector.tensor_tensor(tmp[:], x_sl, ps_b[:, 0:n], op=OP.subtract)
        nc.vector.tensor_tensor(xo[:], tmp[:], ps_b[:, n:2 * n], op=OP.mult)
        return xo

    # =======================================================================
    # main loop over supergroups
    # =======================================================================
    big = ctx.enter_context(tc.tile_pool(name="big", bufs=2))
    graw = ctx.enter_context(tc.tile_pool(name="graw", bufs=2))
    st6p = ctx.enter_context(tc.tile_pool(name="st6p", bufs=2))

    nrm_tiles = {}
    NB8 = SG // 8

    def norm_sg(sg):
        """Normalize enc_out for one supergroup; stage-sweeped per 8 b."""
        b0 = sg * SG
        nrmN = big.tile([P, SG, NCH, E], BF16, tag="nrmN", name="nrmN")
        nrmT = big.tile([P, SG, T], BF16, tag="nrmT", name="nrmT")
        nrm_tiles[sg] = (nrmN, nrmT)
        for g in range(NB8):
            gb = g * 8
            encR = graw.tile([P, 8, NCH, E], BF16, tag="encR", name="encR")
            nc.gpsimd.dma_start(
                out=encR[:],
                in_=ins["enc_out"][b0 + gb:b0 + gb + 8].rearrange(
                    "b (c p) e -> p b c e", p=P))
            st6 = st6p.tile([P, 8, NCH, 6], F32, tag="st6", name="st6")
            for bl in range(8):
                for c in range(NCH):
                    nc.vector.bn_stats(st6[:, bl, c, :], encR[:, bl, c, :])
            # combine even/odd lane stats: mean=(m0+m1)/2,
            # vh=(var+eps)/2=(cv0+cv1)/(2*128) + ((m0-m1)/2)^2/2 + eps/2
            nst = 8 * NCH
            sview = st6[:].rearrange("p b c s -> p (b c) s")

            def sl(k):
                return tap(sview, k, [[6, nst]])

            mcol = st6p.tile([P, 8, NCH], F32, tag="mcol", name="mcol")
            rstd = st6p.tile([P, 8, NCH], F32, tag="rstd", name="rstd")
            dtmp = st6p.tile([P, nst], F32, tag="dtmp", name="dtmp")
            vtmp = st6p.tile([P, nst], F32, tag="vtmp", name="vtmp")
            mv = mcol[:].rearrange("p b c -> p (b c)")
            rv = rstd[:].rearrange("p b c -> p (b c)")
            nc.vector.tensor_tensor(mv, sl(1), sl(4), op=OP.add)
            nc.vector.tensor_scalar(mv, mv, 0.5, None, op0=OP.mult)
            nc.vector.tensor_tensor(dtmp[:], sl(1), sl(4), op=OP.subtract)
            nc.vector.tensor_mul(dtmp[:], dtmp[:], dtmp[:])
            nc.vector.tensor_tensor(vtmp[:], sl(2), sl(5), op=OP.add)
            nc.vector.tensor_scalar(vtmp[:], vtmp[:], 0.5 / E, EPS * 0.5,
                                    op0=OP.mult, op1=OP.add)
            nc.vector.scalar_tensor_tensor(vtmp[:], dtmp[:], 0.125, vtmp[:],
                                           op0=OP.mult, op1=OP.add)
            rsqrt_nr(vtmp[:], P, nst, "ns", out_sl=rv)
            # negmr = -mean*rstd lets ScalarE normalize one chunk per b via
            # its free affine (out = Identity(x*rstd + (-mean*rstd))),
            # unloading the Vector engine (the busiest).
            negmr = st6p.tile([P, 8, NCH], F32, tag="negmr", name="negmr")
            nc.vector.scalar_tensor_tensor(
                negmr[:].rearrange("p b c -> p (b c)"), mv, -1.0, rv,
                op0=OP.mult, op1=OP.mult)
            for bl in range(8):
                b = gb + bl
                for c in range(3):
                    nc.vector.tensor_scalar(
                        nrmN[:, b, c, :], encR[:, bl, c, :],
                        mcol[:, bl, c:c + 1], rstd[:, bl, c:c + 1],
                        op0=OP.subtract, op1=OP.mult)
                nc.scalar.activation(
                    nrmN[:, b, 3, :], encR[:, bl, 3, :], AF.Identity,
                    bias=negmr[:, bl, 3:4], scale=rstd[:, bl, 3:4])
            for bl in range(8):
                b = gb + bl
                ps_t = psum(p_tr, [P, NCH, P])
                for c in range(NCH):
                    nc.tensor.matmul(ps_t[:, c, :], nrmN[:, b, c, :],
                                     id_bf[:])
                if bl % 2 == 0:
                    nc.scalar.copy(nrmT[:, b, :],
                                   ps_t[:].rearrange("p c q -> p (c q)"))
                else:
                    nc.vector.tensor_copy(
                        nrmT[:, b, :], ps_t[:].rearrange("p c q -> p (c q)"))

    norm_sg(0)
    for sg in range(NSG):
        if sg + 1 < NSG:
            norm_sg(sg + 1)
        nrmN, nrmT = nrm_tiles.pop(sg)
        if stage <= 2:
            continue

        # ---------------- decoder layers ----------------
        b0 = sg * SG
        sl_q = q_T[:, b0:b0 + SG]
        for l in range(L):
            w = W[l]
            # ---- q-side: ln3 -> Q -> U' [e, b, h]
            qx3 = ln_cols(sl_q, SG, out_dt=BF16)
            ps_Q = psum(p_sm, [16, H, SG])
            for h in range(H):
                nc.tensor.matmul(ps_Q[:, h, :],
                                 w["wq_e"][:, 16 * h:16 * h + 16], qx3[:],
                                 start=(h == 0), stop=False)
            # qb bias for all heads in one K=8 matmul against the indicator
            nc.tensor.matmul(ps_Q[:].rearrange("p h s -> p (h s)"),
                             w["qbT"][:], ind8[:].rearrange("p a b -> p (a b)"),
                             start=False, stop=True)
            Q_spl = work.tile([16, H, SG], BF16, tag="Qspl")
            nc.vector.tensor_copy(Q_spl[:].rearrange("p h s -> p (h s)"),
                                  ps_Q[:].rearrange("p h s -> p (h s)"))
            ps_U = psum(p_sm, [P, H, SG])
            for h in range(H):
                nc.tensor.matmul(ps_U[:, h, :], w["wk_spl"][:, h, :],
                                 Q_spl[:, h, :])
            # U' stored b-major [e, b, h] so quad weight slices are contiguous;
            # the copy reads ps_U [e, h, b] with a reordering AP.
            U_sb = work.tile([P, SG, H], BF16, tag="Usb")
            nc.scalar.activation(
                U_sb[:].rearrange("p b h -> p (b h)"),
                tap(ps_U[:], 0, [[1, SG], [SG, H]]),
                AF.Copy, scale=w["g1"][:])

            if stage <= 3:
                continue

            # ---- attention: 8 quads of 4 b, software-pipelined by stage so
            # each engine's FIFO queue never blocks on another engine's
            # in-flight work.
            NQ = SG // 4
            rnT_all = work.tile([P, SG, H], BF16, tag="rnT")
            qt = [dict() for _ in range(NQ)]

            def st_scores(q):
                ps_s = psum(p_sc, [P, T])
                qt[q]["ps_s"] = ps_s
                for m in range(4):
                    nc.tensor.matmul(
                        ps_s[32 * m:32 * m + 32, :],
                        U_sb[:, 4 * q:4 * q + 4, :], nrmT[:, 4 * q + m, :],
                        tile_position=(0, 32 * m))

            def st_exp(q):
                expw = work.tile([P, T], BF16, tag="expw")
                dsum = dsp.tile([P, 1], F32, tag="dsum")
                qt[q]["expw"] = expw
                qt[q]["dsum"] = dsum
                nc.scalar.activation(expw[:], qt[q]["ps_s"][:], AF.Exp,
                                     scale=float(D ** 0.5),
                                     accum_out=dsum[:])

            def st_wt(q):
                ps_w = psum(p_wt, [P, NCH, P])
                qt[q]["ps_w"] = ps_w
                expw = qt[q]["expw"]
                for c in range(NCH):
                    nc.tensor.matmul(ps_w[:, c, :],
                                     expw[:, c * P:(c + 1) * P], id_bf[:])

            def st_wc(q):
                weiT = work.tile([P, NCH, P], BF16, tag="weiT")
                qt[q]["weiT"] = weiT
                src = qt[q]["ps_w"][:].rearrange("p c q -> p (c q)")
                dst = weiT[:].rearrange("p c q -> p (c q)")
                if q % 2 == 0:
                    nc.scalar.copy(dst, src)
                else:
                    nc.vector.tensor_copy(dst, src)

            def st_val(q):
                ps_v = psum(p_sm, [P, E])
                qt[q]["ps_v"] = ps_v
                weiT = qt[q]["weiT"]
                for c in range(NCH):
                    for m in range(4):
                        nc.tensor.matmul(
                            ps_v[32 * m:32 * m + 32, :],
                            weiT[:, c, 32 * m:32 * m + 32],
                            nrmN[:, 4 * q + m, c, :],
                            tile_position=(0, 32 * m),
                            start=(c == 0), stop=(c == NCH - 1))

            def st_fix(q):
                ps_v = qt[q]["ps_v"]
                dinv = work.tile([P, 1], F32, tag="dinv")
                nc.vector.reciprocal(dinv[:], qt[q]["dsum"][:])
                rn_q = work.tile([P, E], BF16, tag="rnq")
                qt[q]["rn_q"] = rn_q
                nc.scalar.activation(rn_q[:], ps_v[:], AF.Identity,
                                     scale=dinv[:])

            def st_rnt(q):
                ps_r = psum(p_tr, [P, P])
                qt[q]["ps_r"] = ps_r
                nc.tensor.matmul(ps_r[:], qt[q]["rn_q"][:], id_bf[:])

            def st_gat(q):
                nc.scalar.copy(
                    rnT_all[:, 4 * q:4 * q + 4, :],
                    tap(qt[q]["ps_r"][:], 0, [[40, 4], [1, 8]]))
                qt[q].clear()

            stages = [st_scores, st_exp, st_wt, st_wc, st_val, st_fix,
                      st_rnt, st_gat]
            for step in range(NQ + len(stages) - 1):
                for si in range(len(stages) - 1, -1, -1):
                    q = step - si
                    if 0 <= q < NQ:
                        stages[si](q)

            # ---- att @ wv, then proj back to residual
            ps_at = psum(p_sm, [16, H, SG])
            for h in range(H):
                nc.tensor.matmul(ps_at[:, h, :],
                                 w["wv_bf"][:, 16 * h:16 * h + 16],
                                 rnT_all[:, :, h])
            att_sb = work.tile([16, H, SG], BF16, tag="attsb")
            nc.scalar.copy(att_sb[:].rearrange("p h b -> p (h b)"),
                           ps_at[:].rearrange("p h b -> p (h b)"))
            ps_p = psum(p_sm, [P, SG])
            for h in range(H):
                nc.tensor.matmul(ps_p[:], w["pj_bf"][:, h, :], att_sb[:, h, :],
                                 start=(h == 0), stop=(h == H - 1))
            nc.vector.scalar_tensor_tensor(sl_q, ps_p[:], w["bias2"][:], sl_q,
                                           op0=OP.add, op1=OP.add)

            if stage <= 4:
                continue

            # ---- FFN (gelu-tanh evaluated on the full [P, 4*SG] batch;
            # f1 bias added via a K=4 indicator matmul, 0.5 folded into f2,
            # and gelu split as (f2/2)@g + (f2/2)@(g*tanh))
            qx4 = ln_cols(sl_q, SG, out_dt=BF16)
            ps_h = psum(p_sm, [P, 4, SG])
            ps_hv = ps_h[:].rearrange("p m s -> p (m s)")
            for m in range(4):
                nc.tensor.matmul(ps_h[:, m, :],
                                 w["f1_e"][:, m * E:(m + 1) * E], qx4[:],
                                 start=(m == 0), stop=False)
            nc.tensor.matmul(ps_hv, w["fb1eT"][:],
                             ind8[0:4, 0:4, :].rearrange("p a b -> p (a b)"),
                             start=False, stop=True)
            g_bf = work.tile([P, 4, SG], BF16, tag="gbf")
            nc.vector.tensor_copy(g_bf[:].rearrange("p m s -> p (m s)"), ps_hv)
            g2 = work.tile([P, 4 * SG], F32, tag="gx2")
            # square on DVE (bf16 2x) keeps ScalarE's strict-FIFO queue free
            # for the attention exps
            gbv = g_bf[:].rearrange("p m s -> p (m s)")
            nc.vector.tensor_mul(g2[:], gbv, gbv)
            nc.vector.tensor_scalar(g2[:], g2[:], 0.044715, 1.0,
                                    op0=OP.mult, op1=OP.add)
            nc.vector.tensor_tensor(g2[:], g2[:], ps_hv, op=OP.mult)
            th = work.tile([P, 4, SG], BF16, tag="gth")
            nc.scalar.activation(th[:].rearrange("p m s -> p (m s)"), g2[:],
                                 AF.Tanh, scale=0.7978845608028654)
            gt = work.tile([P, 4, SG], BF16, tag="hgb")
            nc.vector.tensor_mul(gt[:].rearrange("p m s -> p (m s)"),
                                 g_bf[:].rearrange("p m s -> p (m s)"),
                                 th[:].rearrange("p m s -> p (m s)"))
            ps_f = psum(p_sm, [P, SG])
            for k in range(4):
                nc.tensor.matmul(ps_f[:], w["f2_bf"][:, k, :], g_bf[:, k, :],
                                 start=(k == 0), stop=False)
                nc.tensor.matmul(ps_f[:], w["f2_bf"][:, k, :], gt[:, k, :],
                                 start=False, stop=(k == 3))
            nc.vector.scalar_tensor_tensor(sl_q, ps_f[:], w["fb2"][:], sl_q,
                                           op0=OP.add, op1=OP.add)

    if stage <= 5:
        dbg_out(q_T)
        return

    # ---------------- final layernorm + head ----------------
    # Output is produced batch-major ([b, 37] with b on partitions) so the
    # store DMA writes contiguous 148B rows instead of 9472 4-byte packets.
    hw_t = wts.tile([E, 37], F32, tag="hw")
    dma(out=hw_t[:], in_=ins["head_w"])
    gf = load_col("lnf_g", None, P, "gf")
    bf = load_col("lnf_b", None, P, "bf")
    hb_row = wts.tile([1, 37], F32, tag="hb")
    dma(out=hb_row[:], in_=ins["head_b"])
    hw_e = wts.tile([E, 37], BF16, tag="hwe")
    nc.vector.tensor_scalar_mul(hw_e[:], hw_t[:], gf[:])
    ps4 = psum(p_sm, [1, 37])
    nc.tensor.matmul(ps4[:], bf[:], hw_t[:])
    hbe_row = wts.tile([1, 37], F32, tag="hbe")
    nc.vector.tensor_add(hbe_row[:], ps4[:], hb_row[:])
    qxf = ln_cols(q_T[:], BC, out_dt=BF16)
    out_sb = work.tile([P, 2, 37], F32, tag="osb")
    for k in range(2):
        ps_o = psum(p_sc, [P, 37])
        nc.tensor.matmul(ps_o[:], qxf[:, k * P:(k + 1) * P], hw_e[:],
                         start=True, stop=False)
        nc.tensor.matmul(ps_o[:], ones_row[:], hbe_row[:],
                         start=False, stop=True)
        nc.vector.tensor_copy(out_sb[:, k, :], ps_o[:])
    dma(out=out_ap.rearrange("(k p) o -> p k o", p=P), in_=out_sb[:])


def input_specs_for(BC):
    full = [
        ("enc_out", [BC, T, E]), ("x1", [BC, 3, 37, 1]), ("x2", [BC, 7, 4, 1]),
        ("x3", [BC, 4]),
        ("c11_w", [8, 3, 3, 3]), ("c11_b", [8]), ("bn11_g", [8]), ("bn11_b", [8]),
        ("c12_w", [8, 8, 3, 3]), ("c12_b", [8]), ("bn12_g", [8]), ("bn12_b", [8]),
        ("fc1_w", [296, 64]), ("fc1_b", [64]),
        ("c21_w", [8, 7, 3, 3]), ("c21_b", [8]), ("bn21_g", [8]), ("bn21_b", [8]),
        ("fc2_w", [32, 16]), ("fc2_b", [16]),
        ("fc_w", [84, 128]), ("fc_b", [128]),
        ("wk", [L, E, E]), ("wq", [L, E, E]), ("wv", [L, E, E]),
        ("proj_w", [L, E, E]), ("proj_b", [L, E]),
        ("ln1_g", [L, E]), ("ln1_b", [L, E]), ("ln2_g", [L, E]), ("ln2_b", [L, E]),
        ("ln3_g", [L, E]), ("ln3_b", [L, E]), ("ln4_g", [L, E]), ("ln4_b", [L, E]),
        ("ff_w1", [L, E, 4 * E]), ("ff_b1", [L, 4 * E]),
        ("ff_w2", [L, 4 * E, E]), ("ff_b2", [L, E]),
        ("lnf_g", [E]), ("lnf_b", [E]), ("head_w", [E, 37]), ("head_b", [37]),
    ]
    return [(n, s, F32) for n, s in full]


def build_program(BC=256, stage=99):
    nc = bacc.Bacc("TRN2", target_bir_lowering=False, debug=False,
                   enable_asserts=True, num_devices=1)
    ins = {}
    for name, shape, dt_ in input_specs_for(BC):
        ins[name] = nc.dram_tensor(name, shape, dt_, kind="ExternalInput").ap()
    out_ap = nc.dram_tensor("out", [BC, 37], F32, kind="ExternalOutput").ap()
    with tile.TileContext(nc) as tc:
        with ExitStack() as ctx:
            decoder_body(ctx, tc, out_ap, ins, BC, stage=stage)
    nc.compile()
    return nc


_prog_cache = {}


def kernel(**inputs):
    BC = B_FULL // N_CORES
    if BC not in _prog_cache:
        _prog_cache[BC] = build_program(BC)
    nc = _prog_cache[BC]
    in_maps = []
    for c in range(N_CORES):
        m = {}
        for name, shape, _ in input_specs_for(BC):
            arr = np.ascontiguousarray(np.asarray(inputs[name], dtype=np.float32))
            if name in SHARDED:
                arr = arr[c * BC:(c + 1) * BC]
            m[name] = np.ascontiguousarray(arr)
        in_maps.append(m)
    res = run_bass_kernel_spmd(nc, in_maps, core_ids=list(range(N_CORES)))
    return np.concatenate([r["out"] for r in res.results], axis=0)



# revision 52
# speedup vs baseline: 1.2659x; 1.0046x over previous
"""Trainium2 Bass kernel for nn_Decoder (dense_transformer) — v3.

v3 changes vs v2 (2038us -> ~1390us measured, rel err 5.2e-3):
  - No ScalarE Sqrt anywhere: rstd comes from an all-DVE Newton rsqrt
    (bit-trick seed on (var+eps)/2, one tuned iteration). This keeps the
    activation table pinned on exp_and_others (exp/tanh/square/identity),
    eliminating ~122 ACT_TABLE_LOADs (156us of ScalarE).
  - Head emits the output batch-major via two qxf-stationary matmuls +
    a rank-1 bias matmul, so the store DMA writes 148B rows instead of
    9473 4-byte packets (was a 123us serial tail).
  - FFN gelu evaluated once per (sg,l) on [128, 4*SG] with the f1 bias
    added by a K=4 indicator matmul; gelu split as (f2/2)@g + (f2/2)@(g*t)
    with 0.5 folded into the f2 weights.
  - Q-path biases via one K=8 indicator matmul; Q_spl is a single cast.
    wq_e/qx3 in bf16 (fp32 matmuls run LOW/HIGH double-pass on PE).
  - Softmax denominators from the exp activation's accum_out (the ones
    column in the values rhs is gone); st_fix normalize runs on ScalarE.
  - One of four normalize-apply chunks per b runs on ScalarE via its
    free affine (scale=rstd, bias=-mean*rstd).

Key restructuring vs v1 baseline (1532us):
  - Attention matmuls are PE-efficient: scores stream nrm_T (N=512) against a
    32-col stationary U' slice, 4 batch elements packed into one PSUM bank via
    tile_position col-tiling; values stream nrm_nat (N=129) against 32-col
    transposed-softmax weights. No more 128-col weight reloads per (b,chunk).
  - The softmax denominator comes free from an appended ones-column in the
    values rhs (col 128), so rn = ps[:,0:128] * recip(ps[:,128]).
  - All transposes are regular matmuls with a bf16 identity rhs (keeps the PE
    HAM-warm at 2.4GHz, unlike transpose-mode).
  - enc_out is cast fp32->bf16 in the DMA (SWDGE), halving DVE stats/apply
    cost; bn_stats runs per-b (4 chunks in one op) and the mean/var
    aggregation is done with a handful of [128,64] strided ops per group.
  - Layer weights are loaded + folded ONCE (not per supergroup).

Sharding: pure data parallel, batch 2048 -> 8 cores x 256.
"""

import math
from contextlib import ExitStack

import numpy as np

import concourse.bass as bass
import concourse.tile as tile
from concourse import bacc, mybir
from concourse.bass_utils import run_bass_kernel_spmd
from concourse.masks import make_identity

F32 = mybir.dt.float32
BF16 = mybir.dt.bfloat16
I32 = mybir.dt.int32
AF = mybir.ActivationFunctionType
OP = mybir.AluOpType

RSQRT_MAGIC_H = 0x5EF759DF  # quake magic 0x5f3759df shifted for vh = v/2 seed

P = 128
T = 512
E = 128
H = 8
D = 16
L = 3
NCH = T // P            # 4 t-chunks
BN_S = 1.0 / math.sqrt(1.0 + 1e-5)
EPS = 1e-5
N_CORES = 8
B_FULL = 2048
SG = 32                 # supergroup batch size (residual-stream width)
# softmax denominators come from the exp activation's accum_out, so the
# values rhs is just the E normalized columns (no appended ones column).

SHARDED = ("enc_out", "x1", "x2", "x3")


def _ap(t, offset, pattern):
    return bass.AP(tensor=t.tensor, offset=offset, ap=[list(p) for p in pattern])


def tap(ap, extra_off, free_pattern):
    """Sub-AP of a tile AP: keep partition dim, replace free dims."""
    return bass.AP(tensor=ap.tensor, offset=ap.offset + extra_off,
                   ap=[list(ap.ap[0])] + [list(p) for p in free_pattern])


def statenet(ctx, tc, ins, q_T, BC, p_a, p_b):
    """Conv/FC front-end producing q0 [E, BC] into q_T. (v1 logic verbatim.)"""
    nc = tc.nc
    dma = nc.sync.dma_start
    NB = (BC + P - 1) // P

    def psum(pool, shape, dt_=F32):
        return pool.tile(shape, dt_, tag=pool.name, name=pool.name + "_t")

    id_f32 = ctx._id_f32
    id_bf = ctx._id_bf

    # conv/fc tensors are bf16 so the PE runs single-pass matmuls
    # (fp32 operands lower to LOW/HIGH double matmuls).
    with tc.tile_pool(name="snet", bufs=1) as sn:
        x1T = sn.tile([111, BC], BF16)
        x2T = sn.tile([28, BC], BF16)
        cat64 = sn.tile([64, BC], BF16)
        cat16 = sn.tile([16, BC], BF16)
        x3c = sn.tile([4, BC], BF16)
        x3T = x3c[0:4, :]
        x1_f = ins["x1"].rearrange("b c h w -> b (c h w)")
        x2_f = ins["x2"].rearrange("b c h w -> b (c h w)")
        for i in range(NB):
            n = min(P, BC - i * P)
            for (srcx, dstT, w) in ((x1_f, x1T[:], 111), (x2_f, x2T[:], 28),
                                    (ins["x3"], x3T, 4)):
                xin = sn.tile([P, w], F32, tag="xin")
                dma(out=xin[:n, :], in_=srcx[i * P:i * P + n, :])
                pst = psum(p_a, [w, P])
                nc.tensor.transpose(pst[:, :n], xin[:n, :], id_f32[:n, :n])
                nc.scalar.copy(dstT[:, i * P:i * P + n], pst[:, :n])

        def conv_w(dram_ap, O_, C_, gname, bname, cbname):
            KK = C_ * 3
            ws = sn.tile([O_, KK], F32, tag="ws" + gname)
            dma(out=ws[:], in_=_ap(dram_ap, 1, [[C_ * 9, O_], [9, C_], [3, 3]]))
            g = sn.tile([O_, 1], F32, tag="g" + gname)
            dma(out=g[:], in_=ins[gname])
            gp = sn.tile([O_, 1], F32, tag="gp" + gname)
            nc.scalar.mul(gp[:], g[:], BN_S)
            cb = sn.tile([O_, 1], F32, tag="cb" + gname)
            dma(out=cb[:], in_=ins[cbname])
            bb = sn.tile([O_, 1], F32, tag="bb" + gname)
            dma(out=bb[:], in_=ins[bname])
            beff = sn.tile([O_, 1], F32, tag="be" + gname)
            nc.vector.tensor_mul(beff[:], cb[:], gp[:])
            nc.vector.tensor_add(beff[:], beff[:], bb[:])
            wsc = sn.tile([O_, KK], BF16, tag="wsc" + gname)
            nc.vector.tensor_scalar_mul(wsc[:], ws[:], gp[:])
            pswt = psum(p_a, [KK, O_])
            nc.tensor.matmul(pswt[:], wsc[:], id_bf[:O_, :O_])
            wT = sn.tile([KK, O_], BF16, tag="wT" + gname)
            nc.scalar.copy(wT[:], pswt[:])
            return wT, beff

        w1T, b1e = conv_w(ins["c11_w"], 8, 3, "bn11_g", "bn11_b", "c11_b")
        w2T, b2e = conv_w(ins["c12_w"], 8, 8, "bn12_g", "bn12_b", "c12_b")
        w3T, b3e = conv_w(ins["c21_w"], 8, 7, "bn21_g", "bn21_b", "c21_b")

        def im2col(srcT, C_, W_):
            rhs = sn.tile([24, 37, BC], BF16, tag="im", name="imt")[:C_ * 3, :W_, :]
            nc.vector.memset(rhs[:], 0.0)
            for c in range(C_):
                for kh in range(3):
                    lo = max(0, 1 - kh)
                    hi = min(W_, W_ + 1 - kh)
                    n = hi - lo
                    s0 = c * W_ + lo + kh - 1
                    k_ = c * 3 + kh
                    dma(out=rhs[k_:k_ + 1, lo:hi, :], in_=srcT[s0:s0 + n, :])
            return rhs

        def conv_apply(rhs, wT, beff, O_, W_):
            y = sn.tile([8, 37, BC], BF16, tag="yt", name="ytt")[:O_, :W_, :]
            step = max(1, 512 // BC)
            for i0 in range(0, W_, step):
                n = min(step, W_ - i0)
                psc = psum(p_b, [O_, step, BC])
                nc.tensor.matmul(psc[:, :n, :], wT[:], rhs[:, i0:i0 + n, :])
                nc.scalar.activation(y[:, i0:i0 + n, :], psc[:, :n, :],
                                     AF.Relu, bias=beff[:])
            return y

        r9 = im2col(x1T, 3, 37)
        y1 = conv_apply(r9, w1T, b1e, 8, 37)
        r24 = sn.tile([24, 37, BC], BF16, tag="im", name="imt")
        nc.vector.memset(r24[:], 0.0)
        for c in range(8):
            for kh in range(3):
                lo = max(0, 1 - kh)
                hi = min(37, 37 + 1 - kh)
                n = hi - lo
                k_ = c * 3 + kh
                dma(out=r24[k_:k_ + 1, lo:hi, :],
                    in_=y1[c:c + 1, lo + kh - 1:lo + kh - 1 + n, :])
        y2 = conv_apply(r24, w2T, b2e, 8, 37)

        r21 = im2col(x2T, 7, 4)
        y2b = conv_apply(r21, w3T, b3e, 8, 4)

        y2r = []
        for k, (ilo, ihi) in enumerate(((0, 16), (16, 32), (32, 37))):
            ni = ihi - ilo
            t_ = sn.tile([ni * 8, BC], BF16, tag=f"y2r{k}")
            for o in range(8):
                dma(out=t_[o * ni:(o + 1) * ni, :], in_=y2[o:o + 1, ilo:ihi, :])
            y2r.append(t_)
        y2br = sn.tile([32, BC], BF16)
        for o in range(8):
            dma(out=y2br[o * 4:(o + 1) * 4, :], in_=y2b[o:o + 1, :, :])

        ps_h1 = psum(p_b, [64, BC])
        for k, (ilo, ihi) in enumerate(((0, 16), (16, 32), (32, 37))):
            ni = ihi - ilo
            fw = sn.tile([ni * 8, 64], BF16, tag=f"fw{k}")
            nc.gpsimd.dma_start(out=fw[:], in_=_ap(ins["fc1_w"], ilo * 64,
                                [[37 * 64, 8], [64, ni], [1, 64]]))
            nc.tensor.matmul(ps_h1[:], fw[:], y2r[k][:],
                             start=(k == 0), stop=(k == 2))
        fb1 = sn.tile([64, 1], F32)
        dma(out=fb1[:], in_=ins["fc1_b"])
        h1 = cat64[0:64, :]
        nc.scalar.activation(h1, ps_h1[:], AF.Relu, bias=fb1[:])

        fw2 = sn.tile([32, 16], BF16)
        nc.gpsimd.dma_start(out=fw2[:], in_=ins["fc2_w"])
        ps_h2 = psum(p_b, [16, BC])
        nc.tensor.matmul(ps_h2[:], fw2[:], y2br[:])
        fb2 = sn.tile([16, 1], F32)
        dma(out=fb2[:], in_=ins["fc2_b"])
        h2 = cat16[0:16, :]
        nc.scalar.activation(h2, ps_h2[:], AF.Relu, bias=fb2[:])

        fcw64 = sn.tile([64, E], BF16)
        nc.gpsimd.dma_start(out=fcw64[:], in_=ins["fc_w"][0:64, :])
        fcw16 = sn.tile([16, E], BF16)
        nc.gpsimd.dma_start(out=fcw16[:], in_=ins["fc_w"][64:80, :])
        fcw3 = sn.tile([4, E], BF16)
        nc.gpsimd.dma_start(out=fcw3[:], in_=ins["fc_w"][80:84, :])
        ps_q0 = psum(p_b, [P, BC])
        nc.tensor.matmul(ps_q0[:], fcw64[:], cat64[:], start=True, stop=False)
        nc.tensor.matmul(ps_q0[:], fcw16[:], cat16[:], start=False, stop=False)
        nc.tensor.matmul(ps_q0[:], fcw3[:], x3c[:], start=False, stop=True)
        fcb = sn.tile([P, 1], F32)
        dma(out=fcb[:], in_=ins["fc_b"])
        nc.scalar.activation(q_T[:], ps_q0[:], AF.Relu, bias=fcb[:])


def decoder_body(ctx: ExitStack, tc: tile.TileContext, out_ap: bass.AP,
                 ins: dict, BC: int, stage: int = 99):
    nc = tc.nc
    dma = nc.sync.dma_start
    NSG = BC // SG

    def dbg_out(tag_ap):
        nc.sync.dma_start(out=out_ap.rearrange("b o -> o b"),
                          in_=tag_ap[0:37, 0:BC])

    # ---------------- pools ----------------
    const = ctx.enter_context(tc.tile_pool(name="const", bufs=1))
    wts = ctx.enter_context(tc.tile_pool(name="wts", bufs=1))
    perm = ctx.enter_context(tc.tile_pool(name="perm", bufs=1))
    # PSUM: 8 banks; each pool = bufs x max-2KB tile
    p_sc = ctx.enter_context(tc.tile_pool(name="p_sc", bufs=2, space="PSUM"))
    p_tr = ctx.enter_context(tc.tile_pool(name="p_tr", bufs=2, space="PSUM"))
    p_wt = ctx.enter_context(tc.tile_pool(name="p_wt", bufs=2, space="PSUM"))
    p_sm = ctx.enter_context(tc.tile_pool(name="p_sm", bufs=2, space="PSUM"))
    # softmax denominators live 4 pipeline steps (st_exp -> st_fix); a 2-slot
    # tag would stall st_exp(q) on st_fix(q-2), collapsing the attention
    # pipeline depth. Tiny tiles, so give them a deep dedicated pool.
    dsp = ctx.enter_context(tc.tile_pool(name="dsp", bufs=8))

    def psum(pool, shape, dt_=F32):
        return pool.tile(shape, dt_, tag=pool.name, name=pool.name + "_t")

    id_f32 = const.tile([P, P], F32)
    id_bf = const.tile([P, P], BF16)
    make_identity(nc, id_f32[:])
    make_identity(nc, id_bf[:])
    ctx._id_f32 = id_f32
    ctx._id_bf = id_bf
    ones_col = const.tile([P, 1], F32)
    nc.vector.memset(ones_col[:], 1.0)
    ones_row = const.tile([1, P], F32)
    nc.vector.memset(ones_row[:], 1.0)
    # Newton-rsqrt constants (all-DVE rstd; keeps Sqrt off ScalarE so the
    # activation table never swaps away from exp_and_others). Single tuned
    # Newton step y0*(A - B*vh*y0^2): max rel err 8.8e-4.
    magic_t = const.tile([P, BC], I32)
    nc.vector.memset(magic_t[:], RSQRT_MAGIC_H)
    ca_t = const.tile([P, BC], F32)
    nc.vector.memset(ca_t[:], 1.50133365)
    # ind8[h, h'*32+j] = (h == h'): K=8 indicator used to add per-head/-chunk
    # biases with a single accumulating matmul (rows 0:4, cols 0:128 double as
    # the K=4 FFN-bias indicator).
    ind8 = const.tile([8, 8, SG], BF16)
    ones_row_bf = const.tile([1, SG], BF16)
    nc.vector.memset(ones_row_bf[:], 1.0)
    nc.vector.memset(ind8[:], 0.0)
    for hh in range(8):
        nc.sync.dma_start(out=ind8[hh:hh + 1, hh, :], in_=ones_row_bf[:])

    def rsqrt_nr(vh_sl, p_, n_, tagp, out_sl=None):
        """rstd = 1/sqrt(2*vh) via bit-trick seed + 2 Newton iters (DVE only).

        vh_sl: [p_, n_] f32 AP holding (var + eps) / 2. Writes into out_sl
        if given (returns it), else into a scratch tile."""
        ti = work.tile([p_, n_], I32, tag=tagp + "ti", name=tagp + "ti")
        y = work.tile([p_, n_], F32, tag=tagp + "y", name=tagp + "y")
        t = work.tile([p_, n_], F32, tag=tagp + "t", name=tagp + "t")
        nc.vector.tensor_scalar(ti[:], vh_sl.bitcast(I32), 1, None,
                                op0=OP.logical_shift_right)
        nc.vector.tensor_tensor(y[:].bitcast(I32), magic_t[0:p_, 0:n_],
                                ti[:], op=OP.subtract)
        nc.vector.tensor_mul(t[:], y[:], y[:])
        nc.vector.tensor_tensor(t[:], vh_sl, t[:], op=OP.mult)
        nc.vector.scalar_tensor_tensor(t[:], t[:], -1.00091486,
                                       ca_t[0:p_, 0:n_],
                                       op0=OP.mult, op1=OP.add)
        dst = y[:] if out_sl is None else out_sl
        nc.vector.tensor_tensor(dst, y[:], t[:], op=OP.mult)
        return y[:] if out_sl is None else out_sl

    q_T = perm.tile([P, BC], F32)            # persistent residual [E, b]

    # =======================================================================
    # StateNet (scoped; its SBUF is reclaimed before the big pools open)
    # =======================================================================
    statenet(ctx, tc, ins, q_T, BC, p_sm, p_sc)
    if stage <= 1:
        dbg_out(q_T)
        return

    # =======================================================================
    # Phase 0: load + fold all layer weights once
    # =======================================================================
    def load_col(name, l, n, tg):
        t_ = wts.tile([n, 1], F32, tag=tg)
        src = ins[name]
        dma(out=t_[:], in_=src[l] if l is not None else src)
        return t_

    W = []  # per-layer dict of folded weights (raw loads live in a scope)
    with tc.tile_pool(name="wraw", bufs=1) as wr:
        for l in range(L):
            w = {}
            wq_t = wr.tile([E, E], F32, tag="wq")
            dma(out=wq_t[:], in_=ins["wq"][l])
            wk_t = wr.tile([E, E], F32, tag="wk", name="wk_t")
            dma(out=wk_t[:], in_=ins["wk"][l])
            pj_t = wr.tile([E, E], F32, tag="pj", name="pj_t")
            dma(out=pj_t[:], in_=ins["proj_w"][l])
            wv_t = wr.tile([E, E], F32, tag="wv", name="wv_t")
            dma(out=wv_t[:], in_=ins["wv"][l])
            g1 = load_col("ln1_g", l, P, f"g1{l}")
            g2 = wr.tile([P, 1], F32, tag="g2")
            dma(out=g2[:], in_=ins["ln2_g"][l])
            b2 = wr.tile([P, 1], F32, tag="b2")
            dma(out=b2[:], in_=ins["ln2_b"][l])
            g3 = wr.tile([P, 1], F32, tag="g3")
            dma(out=g3[:], in_=ins["ln3_g"][l])
            b3 = load_col("ln3_b", l, P, f"b3{l}")
            g4 = wr.tile([P, 1], F32, tag="g4")
            dma(out=g4[:], in_=ins["ln4_g"][l])
            b4 = load_col("ln4_b", l, P, f"b4{l}")
            pjb = wr.tile([P, 1], F32, tag="pjb")
            dma(out=pjb[:], in_=ins["proj_b"][l])
            w["g1"], w["b3"], w["b4"] = g1, b3, b4
            w["fb2"] = load_col("ff_b2", l, P, f"fb2{l}")

            wq_e = wts.tile([E, E], BF16, tag=f"wqe{l}")
            nc.vector.tensor_scalar_mul(wq_e[:], wq_t[:], g3[:])
            w["wq_e"] = wq_e
            qb_ps = psum(p_sm, [16, H])
            for h in range(H):
                nc.tensor.matmul(qb_ps[:, h:h + 1],
                                 wq_t[:, 16 * h:16 * h + 16], b3[:])
            qb_spl = wr.tile([16, H], F32, tag="qb", name="qb_spl")
            nc.scalar.copy(qb_spl[:], qb_ps[:])
            ps_qbT = psum(p_sm, [H, 16])
            nc.tensor.matmul(ps_qbT[:], qb_spl[:], id_f32[0:16, 0:16])
            qbT = wts.tile([H, 16], BF16, tag=f"qbT{l}")
            nc.scalar.copy(qbT[:], ps_qbT[:])
            w["qbT"] = qbT

            wk_spl = wts.tile([16, H, E], BF16, tag=f"wks{l}")
            for hh in range(2):
                ps_kT = psum(p_sm, [16, 4, E])
                for h4 in range(4):
                    h = hh * 4 + h4
                    nc.tensor.transpose(ps_kT[:, h4, :],
                                        wk_t[:, 16 * h:16 * h + 16], id_f32[:])
                nc.scalar.copy(wk_spl[:, 4 * hh:4 * hh + 4, :], ps_kT[:])
            w["wk_spl"] = wk_spl

            wv_e = wr.tile([E, E], F32, tag="wve", name="wv_e")
            nc.vector.tensor_scalar_mul(wv_e[:], wv_t[:], g2[:])
            wv_bf = wts.tile([E, E], BF16, tag=f"wvbf{l}")
            nc.vector.tensor_copy(wv_bf[:], wv_e[:])
            w["wv_bf"] = wv_bf
            ps2 = psum(p_sm, [P, 1])
            nc.tensor.matmul(ps2[:], wv_e[:], b2[:])
            c2 = wr.tile([P, 1], F32, tag="c2", name="c2")
            nc.scalar.copy(c2[:], ps2[:])
            ps2b = psum(p_sm, [P, 1])
            nc.tensor.matmul(ps2b[:], pj_t[:], c2[:])
            bias2 = wts.tile([P, 1], F32, tag=f"bias2{l}")
            nc.vector.tensor_add(bias2[:], ps2b[:], pjb[:])
            w["bias2"] = bias2

            pj_bf = wts.tile([16, H, E], BF16, tag=f"pjs{l}")
            pj_f = wr.tile([16, H, E], F32, tag="pjf", name="pj_f")
            dma(out=pj_f[:], in_=_ap(ins["proj_w"], l * E * E,
                                     [[E, 16], [16 * E, H], [1, E]]))
            nc.vector.tensor_copy(pj_bf[:], pj_f[:])
            w["pj_bf"] = pj_bf

            f1_t = wr.tile([E, 4 * E], F32, tag="f1", name="f1_t")
            dma(out=f1_t[:], in_=ins["ff_w1"][l])
            f1_e = wts.tile([E, 4 * E], BF16, tag=f"f1e{l}")
            f1_ef = wr.tile([E, 4 * E], F32, tag="f1ef", name="f1_ef")
            nc.vector.tensor_scalar_mul(f1_ef[:], f1_t[:], g4[:])
            nc.vector.tensor_copy(f1_e[:], f1_ef[:])
            w["f1_e"] = f1_e
            ps3 = psum(p_sm, [P, 4])
            for m in range(4):
                nc.tensor.matmul(ps3[:, m:m + 1], f1_ef[:, m * E:(m + 1) * E],
                                 b4[:])
            fb1_ = wr.tile([P, 4], F32, tag="fb1", name="fb1_")
            dma(out=fb1_[:], in_=ins["ff_b1"][l].rearrange("(c p) -> p c", p=P))
            fb1e = wr.tile([P, 4], F32, tag="fb1e", name="fb1e")
            nc.vector.tensor_add(fb1e[:], ps3[:], fb1_[:])
            ps_bT = psum(p_sm, [4, P])
            nc.tensor.matmul(ps_bT[:], fb1e[:], id_f32[:])
            fb1eT = wts.tile([4, P], BF16, tag=f"fb1eT{l}")
            nc.scalar.copy(fb1eT[:], ps_bT[:])
            w["fb1eT"] = fb1eT

            f2_f = wr.tile([P, 4, E], F32, tag="f2f", name="f2_f")
            dma(out=f2_f[:],
                in_=ins["ff_w2"][l].rearrange("(c p) e -> p c e", p=P))
            # halved so gelu = (f2/2)@g + (f2/2)@(g*tanh) needs no +1/×0.5 ops
            f2_bf = wts.tile([P, 4, E], BF16, tag=f"f2{l}")
            nc.vector.tensor_scalar_mul(f2_bf[:], f2_f[:], 0.5)
            w["f2_bf"] = f2_bf
            W.append(w)

    # =======================================================================
    # helper: layernorm of feature-major [128, n] slice (stats over
    # partitions via PE ones-matmuls; broadcast back via PE).
    # =======================================================================
    work = ctx.enter_context(tc.tile_pool(name="work", bufs=2))

    def ln_cols(x_sl, n, out_dt=F32):
        sq = work.tile([P, BC], F32, tag="sq", name="sq")[:, :n]
        nc.vector.tensor_mul(sq[:], x_sl, x_sl)
        ps_st = psum(p_sm, [1, 2 * n])
        nc.tensor.matmul(ps_st[:, 0:n], ones_col[:], x_sl)
        nc.tensor.matmul(ps_st[:, n:2 * n], ones_col[:], sq[:])
        mean = work.tile([1, BC], F32, tag="mmr", name="mmr")[:, :n]
        nc.vector.tensor_scalar(mean[:], ps_st[:, 0:n], 1.0 / E, None,
                                op0=OP.mult)
        vh = work.tile([1, BC], F32, tag="var", name="var")[:, :n]
        nc.vector.tensor_scalar(vh[:], ps_st[:, n:2 * n], 0.5 / E, EPS * 0.5,
                                op0=OP.mult, op1=OP.add)
        m2 = work.tile([1, BC], F32, tag="m2r", name="m2r")[:, :n]
        nc.vector.tensor_mul(m2[:], mean[:], mean[:])
        nc.vector.scalar_tensor_tensor(vh[:], m2[:], -0.5, vh[:],
                                       op0=OP.mult, op1=OP.add)
        srt = work.tile([1, BC], F32, tag="srt", name="srt")[:, :n]
        rsqrt_nr(vh[:], 1, n, "lc", out_sl=srt[:])
        ps_b = psum(p_sm, [P, 2 * n])
        nc.tensor.matmul(ps_b[:, 0:n], ones_row[:], mean[:])
        nc.tensor.matmul(ps_b[:, n:2 * n], ones_row[:], srt[:])
        xo = work.tile([P, BC], out_dt, tag="xo" + str(out_dt), name="xo")[:, :n]
        tmp = work.tile([P, BC], F32, tag="xt", name="xt")[:, :n]
        nc.vector.tensor_tensor(tmp[:], x_sl, ps_b[:, 0:n], op=OP.subtract)
        nc.vector.tensor_tensor(xo[:], tmp[:], ps_b[:, n:2 * n], op=OP.mult)
        return xo

    # =======================================================================
    # main loop over supergroups
    # =======================================================================
    big = ctx.enter_context(tc.tile_pool(name="big", bufs=2))
    graw = ctx.enter_context(tc.tile_pool(name="graw", bufs=2))
    # st6p=3: stats tiles (st6/mcol/rstd/negmr) bridge stats->combine->apply;
    # a third slot lets group g+2's stats start before g's apply retires.
    st6p = ctx.enter_context(tc.tile_pool(name="st6p", bufs=3))

    nrm_tiles = {}
    NB8 = SG // 8

    def norm_sg(sg):
        """Normalize enc_out for one supergroup; stage-sweeped per 8 b."""
        b0 = sg * SG
        nrmN = big.tile([P, SG, NCH, E], BF16, tag="nrmN", name="nrmN")
        nrmT = big.tile([P, SG, T], BF16, tag="nrmT", name="nrmT")
        nrm_tiles[sg] = (nrmN, nrmT)
        for g in range(NB8):
            gb = g * 8
            encR = graw.tile([P, 8, NCH, E], BF16, tag="encR", name="encR")
            nc.gpsimd.dma_start(
                out=encR[:],
                in_=ins["enc_out"][b0 + gb:b0 + gb + 8].rearrange(
                    "b (c p) e -> p b c e", p=P))
            st6 = st6p.tile([P, 8, NCH, 6], F32, tag="st6", name="st6")
            for bl in range(8):
                for c in range(NCH):
                    nc.vector.bn_stats(st6[:, bl, c, :], encR[:, bl, c, :])
            # combine even/odd lane stats: mean=(m0+m1)/2,
            # vh=(var+eps)/2=(cv0+cv1)/(2*128) + ((m0-m1)/2)^2/2 + eps/2
            nst = 8 * NCH
            sview = st6[:].rearrange("p b c s -> p (b c) s")

            def sl(k):
                return tap(sview, k, [[6, nst]])

            mcol = st6p.tile([P, 8, NCH], F32, tag="mcol", name="mcol")
            rstd = st6p.tile([P, 8, NCH], F32, tag="rstd", name="rstd")
            dtmp = st6p.tile([P, nst], F32, tag="dtmp", name="dtmp")
            vtmp = st6p.tile([P, nst], F32, tag="vtmp", name="vtmp")
            mv = mcol[:].rearrange("p b c -> p (b c)")
            rv = rstd[:].rearrange("p b c -> p (b c)")
            nc.vector.tensor_tensor(mv, sl(1), sl(4), op=OP.add)
            nc.vector.tensor_scalar(mv, mv, 0.5, None, op0=OP.mult)
            nc.vector.tensor_tensor(dtmp[:], sl(1), sl(4), op=OP.subtract)
            nc.vector.tensor_mul(dtmp[:], dtmp[:], dtmp[:])
            nc.vector.tensor_tensor(vtmp[:], sl(2), sl(5), op=OP.add)
            nc.vector.tensor_scalar(vtmp[:], vtmp[:], 0.5 / E, EPS * 0.5,
                                    op0=OP.mult, op1=OP.add)
            nc.vector.scalar_tensor_tensor(vtmp[:], dtmp[:], 0.125, vtmp[:],
                                           op0=OP.mult, op1=OP.add)
            rsqrt_nr(vtmp[:], P, nst, "ns", out_sl=rv)
            # negmr = -mean*rstd lets ScalarE normalize one chunk per b via
            # its free affine (out = Identity(x*rstd + (-mean*rstd))),
            # unloading the Vector engine (the busiest).
            negmr = st6p.tile([P, 8, NCH], F32, tag="negmr", name="negmr")
            nc.vector.scalar_tensor_tensor(
                negmr[:].rearrange("p b c -> p (b c)"), mv, -1.0, rv,
                op0=OP.mult, op1=OP.mult)
            for bl in range(8):
                b = gb + bl
                for c in range(3):
                    nc.vector.tensor_scalar(
                        nrmN[:, b, c, :], encR[:, bl, c, :],
                        mcol[:, bl, c:c + 1], rstd[:, bl, c:c + 1],
                        op0=OP.subtract, op1=OP.mult)
                nc.scalar.activation(
                    nrmN[:, b, 3, :], encR[:, bl, 3, :], AF.Identity,
                    bias=negmr[:, bl, 3:4], scale=rstd[:, bl, 3:4])
            for bl in range(8):
                b = gb + bl
                ps_t = psum(p_tr, [P, NCH, P])
                for c in range(NCH):
                    nc.tensor.matmul(ps_t[:, c, :], nrmN[:, b, c, :],
                                     id_bf[:])
                if bl % 2 == 0:
                    nc.scalar.copy(nrmT[:, b, :],
                                   ps_t[:].rearrange("p c q -> p (c q)"))
                else:
                    nc.vector.tensor_copy(
                        nrmT[:, b, :], ps_t[:].rearrange("p c q -> p (c q)"))

    norm_sg(0)
    for sg in range(NSG):
        if sg + 1 < NSG:
            norm_sg(sg + 1)
        nrmN, nrmT = nrm_tiles.pop(sg)
        if stage <= 2:
            continue

        # ---------------- decoder layers ----------------
        b0 = sg * SG
        sl_q = q_T[:, b0:b0 + SG]
        for l in range(L):
            w = W[l]
            # ---- q-side: ln3 -> Q -> U' [e, b, h]
            qx3 = ln_cols(sl_q, SG, out_dt=BF16)
            ps_Q = psum(p_sm, [16, H, SG])
            for h in range(H):
                nc.tensor.matmul(ps_Q[:, h, :],
                                 w["wq_e"][:, 16 * h:16 * h + 16], qx3[:],
                                 start=(h == 0), stop=False)
            # qb bias for all heads in one K=8 matmul against the indicator
            nc.tensor.matmul(ps_Q[:].rearrange("p h s -> p (h s)"),
                             w["qbT"][:], ind8[:].rearrange("p a b -> p (a b)"),
                             start=False, stop=True)
            Q_spl = work.tile([16, H, SG], BF16, tag="Qspl")
            nc.vector.tensor_copy(Q_spl[:].rearrange("p h s -> p (h s)"),
                                  ps_Q[:].rearrange("p h s -> p (h s)"))
            ps_U = psum(p_sm, [P, H, SG])
            for h in range(H):
                nc.tensor.matmul(ps_U[:, h, :], w["wk_spl"][:, h, :],
                                 Q_spl[:, h, :])
            # U' stored b-major [e, b, h] so quad weight slices are contiguous;
            # the copy reads ps_U [e, h, b] with a reordering AP.
            U_sb = work.tile([P, SG, H], BF16, tag="Usb")
            nc.scalar.activation(
                U_sb[:].rearrange("p b h -> p (b h)"),
                tap(ps_U[:], 0, [[1, SG], [SG, H]]),
                AF.Copy, scale=w["g1"][:])

            if stage <= 3:
                continue

            # ---- attention: 8 quads of 4 b, software-pipelined by stage so
            # each engine's FIFO queue never blocks on another engine's
            # in-flight work.
            NQ = SG // 4
            rnT_all = work.tile([P, SG, H], BF16, tag="rnT")
            qt = [dict() for _ in range(NQ)]

            def st_scores(q):
                ps_s = psum(p_sc, [P, T])
                qt[q]["ps_s"] = ps_s
                for m in range(4):
                    nc.tensor.matmul(
                        ps_s[32 * m:32 * m + 32, :],
                        U_sb[:, 4 * q:4 * q + 4, :], nrmT[:, 4 * q + m, :],
                        tile_position=(0, 32 * m))

            def st_exp(q):
                expw = work.tile([P, T], BF16, tag="expw")
                dsum = dsp.tile([P, 1], F32, tag="dsum")
                qt[q]["expw"] = expw
                qt[q]["dsum"] = dsum
                nc.scalar.activation(expw[:], qt[q]["ps_s"][:], AF.Exp,
                                     scale=float(D ** 0.5),
                                     accum_out=dsum[:])

            def st_wt(q):
                ps_w = psum(p_wt, [P, NCH, P])
                qt[q]["ps_w"] = ps_w
                expw = qt[q]["expw"]
                for c in range(NCH):
                    nc.tensor.matmul(ps_w[:, c, :],
                                     expw[:, c * P:(c + 1) * P], id_bf[:])

            def st_wc(q):
                weiT = work.tile([P, NCH, P], BF16, tag="weiT")
                qt[q]["weiT"] = weiT
                src = qt[q]["ps_w"][:].rearrange("p c q -> p (c q)")
                dst = weiT[:].rearrange("p c q -> p (c q)")
                if q % 2 == 0:
                    nc.scalar.copy(dst, src)
                else:
                    nc.vector.tensor_copy(dst, src)

            def st_val(q):
                ps_v = psum(p_sm, [P, E])
                qt[q]["ps_v"] = ps_v
                weiT = qt[q]["weiT"]
                for c in range(NCH):
                    for m in range(4):
                        nc.tensor.matmul(
                            ps_v[32 * m:32 * m + 32, :],
                            weiT[:, c, 32 * m:32 * m + 32],
                            nrmN[:, 4 * q + m, c, :],
                            tile_position=(0, 32 * m),
                            start=(c == 0), stop=(c == NCH - 1))

            def st_fix(q):
                ps_v = qt[q]["ps_v"]
                dinv = work.tile([P, 1], F32, tag="dinv")
                nc.vector.reciprocal(dinv[:], qt[q]["dsum"][:])
                rn_q = work.tile([P, E], BF16, tag="rnq")
                qt[q]["rn_q"] = rn_q
                nc.scalar.activation(rn_q[:], ps_v[:], AF.Identity,
                                     scale=dinv[:])

            def st_rnt(q):
                ps_r = psum(p_tr, [P, P])
                qt[q]["ps_r"] = ps_r
                nc.tensor.matmul(ps_r[:], qt[q]["rn_q"][:], id_bf[:])

            def st_gat(q):
                nc.scalar.copy(
                    rnT_all[:, 4 * q:4 * q + 4, :],
                    tap(qt[q]["ps_r"][:], 0, [[40, 4], [1, 8]]))
                qt[q].clear()

            stages = [st_scores, st_exp, st_wt, st_wc, st_val, st_fix,
                      st_rnt, st_gat]
            for step in range(NQ + len(stages) - 1):
                for si in range(len(stages) - 1, -1, -1):
                    q = step - si
                    if 0 <= q < NQ:
                        stages[si](q)

            # ---- att @ wv, then proj back to residual
            ps_at = psum(p_sm, [16, H, SG])
            for h in range(H):
                nc.tensor.matmul(ps_at[:, h, :],
                                 w["wv_bf"][:, 16 * h:16 * h + 16],
                                 rnT_all[:, :, h])
            att_sb = work.tile([16, H, SG], BF16, tag="attsb")
            nc.scalar.copy(att_sb[:].rearrange("p h b -> p (h b)"),
                           ps_at[:].rearrange("p h b -> p (h b)"))
            ps_p = psum(p_sm, [P, SG])
            for h in range(H):
                nc.tensor.matmul(ps_p[:], w["pj_bf"][:, h, :], att_sb[:, h, :],
                                 start=(h == 0), stop=(h == H - 1))
            nc.vector.scalar_tensor_tensor(sl_q, ps_p[:], w["bias2"][:], sl_q,
                                           op0=OP.add, op1=OP.add)

            if stage <= 4:
                continue

            # ---- FFN (gelu-tanh evaluated on the full [P, 4*SG] batch;
            # f1 bias added via a K=4 indicator matmul, 0.5 folded into f2,
            # and gelu split as (f2/2)@g + (f2/2)@(g*tanh))
            qx4 = ln_cols(sl_q, SG, out_dt=BF16)
            ps_h = psum(p_sm, [P, 4, SG])
            ps_hv = ps_h[:].rearrange("p m s -> p (m s)")
            for m in range(4):
                nc.tensor.matmul(ps_h[:, m, :],
                                 w["f1_e"][:, m * E:(m + 1) * E], qx4[:],
                                 start=(m == 0), stop=False)
            nc.tensor.matmul(ps_hv, w["fb1eT"][:],
                             ind8[0:4, 0:4, :].rearrange("p a b -> p (a b)"),
                             start=False, stop=True)
            g_bf = work.tile([P, 4, SG], BF16, tag="gbf")
            nc.vector.tensor_copy(g_bf[:].rearrange("p m s -> p (m s)"), ps_hv)
            g2 = work.tile([P, 4 * SG], F32, tag="gx2")
            # square on DVE (bf16 2x) keeps ScalarE's strict-FIFO queue free
            # for the attention exps
            gbv = g_bf[:].rearrange("p m s -> p (m s)")
            nc.vector.tensor_mul(g2[:], gbv, gbv)
            nc.vector.tensor_scalar(g2[:], g2[:], 0.044715, 1.0,
                                    op0=OP.mult, op1=OP.add)
            nc.vector.tensor_tensor(g2[:], g2[:], ps_hv, op=OP.mult)
            th = work.tile([P, 4, SG], BF16, tag="gth")
            nc.scalar.activation(th[:].rearrange("p m s -> p (m s)"), g2[:],
                                 AF.Tanh, scale=0.7978845608028654)
            gt = work.tile([P, 4, SG], BF16, tag="hgb")
            nc.vector.tensor_mul(gt[:].rearrange("p m s -> p (m s)"),
                                 g_bf[:].rearrange("p m s -> p (m s)"),
                                 th[:].rearrange("p m s -> p (m s)"))
            ps_f = psum(p_sm, [P, SG])
            for k in range(4):
                nc.tensor.matmul(ps_f[:], w["f2_bf"][:, k, :], g_bf[:, k, :],
                                 start=(k == 0), stop=False)
                nc.tensor.matmul(ps_f[:], w["f2_bf"][:, k, :], gt[:, k, :],
                                 start=False, stop=(k == 3))
            nc.vector.scalar_tensor_tensor(sl_q, ps_f[:], w["fb2"][:], sl_q,
                                           op0=OP.add, op1=OP.add)

    if stage <= 5:
        dbg_out(q_T)
        return

    # ---------------- final layernorm + head ----------------
    # Output is produced batch-major ([b, 37] with b on partitions) so the
    # store DMA writes contiguous 148B rows instead of 9472 4-byte packets.
    hw_t = wts.tile([E, 37], F32, tag="hw")
    dma(out=hw_t[:], in_=ins["head_w"])
    gf = load_col("lnf_g", None, P, "gf")
    bf = load_col("lnf_b", None, P, "bf")
    hb_row = wts.tile([1, 37], F32, tag="hb")
    dma(out=hb_row[:], in_=ins["head_b"])
    hw_e = wts.tile([E, 37], BF16, tag="hwe")
    nc.vector.tensor_scalar_mul(hw_e[:], hw_t[:], gf[:])
    ps4 = psum(p_sm, [1, 37])
    nc.tensor.matmul(ps4[:], bf[:], hw_t[:])
    hbe_row = wts.tile([1, 37], F32, tag="hbe")
    nc.vector.tensor_add(hbe_row[:], ps4[:], hb_row[:])
    qxf = ln_cols(q_T[:], BC, out_dt=BF16)
    out_sb = work.tile([P, 2, 37], F32, tag="osb")
    for k in range(2):
        ps_o = psum(p_sc, [P, 37])
        nc.tensor.matmul(ps_o[:], qxf[:, k * P:(k + 1) * P], hw_e[:],
                         start=True, stop=False)
        nc.tensor.matmul(ps_o[:], ones_row[:], hbe_row[:],
                         start=False, stop=True)
        nc.vector.tensor_copy(out_sb[:, k, :], ps_o[:])
    dma(out=out_ap.rearrange("(k p) o -> p k o", p=P), in_=out_sb[:])


def input_specs_for(BC):
    full = [
        ("enc_out", [BC, T, E]), ("x1", [BC, 3, 37, 1]), ("x2", [BC, 7, 4, 1]),
        ("x3", [BC, 4]),
        ("c11_w", [8, 3, 3, 3]), ("c11_b", [8]), ("bn11_g", [8]), ("bn11_b", [8]),
        ("c12_w", [8, 8, 3, 3]), ("c12_b", [8]), ("bn12_g", [8]), ("bn12_b", [8]),
        ("fc1_w", [296, 64]), ("fc1_b", [64]),
        ("c21_w", [8, 7, 3, 3]), ("c21_b", [8]), ("bn21_g", [8]), ("bn21_b", [8]),
        ("fc2_w", [32, 16]), ("fc2_b", [16]),
        ("fc_w", [84, 128]), ("fc_b", [128]),
        ("wk", [L, E, E]), ("wq", [L, E, E]), ("wv", [L, E, E]),
        ("proj_w", [L, E, E]), ("proj_b", [L, E]),
        ("ln1_g", [L, E]), ("ln1_b", [L, E]), ("ln2_g", [L, E]), ("ln2_b", [L, E]),
        ("ln3_g", [L, E]), ("ln3_b", [L, E]), ("ln4_g", [L, E]), ("ln4_b", [L, E]),
        ("ff_w1", [L, E, 4 * E]), ("ff_b1", [L, 4 * E]),
        ("ff_w2", [L, 4 * E, E]), ("ff_b2", [L, E]),
        ("lnf_g", [E]), ("lnf_b", [E]), ("head_w", [E, 37]), ("head_b", [37]),
    ]
    return [(n, s, F32) for n, s in full]


def build_program(BC=256, stage=99):
    nc = bacc.Bacc("TRN2", target_bir_lowering=False, debug=False,
                   enable_asserts=True, num_devices=1)
    ins = {}
    for name, shape, dt_ in input_specs_for(BC):
        ins[name] = nc.dram_tensor(name, shape, dt_, kind="ExternalInput").ap()
    out_ap = nc.dram_tensor("out", [BC, 37], F32, kind="ExternalOutput").ap()
    with tile.TileContext(nc) as tc:
        with ExitStack() as ctx:
            decoder_body(ctx, tc, out_ap, ins, BC, stage=stage)
    nc.compile()
    return nc


_prog_cache = {}


def kernel(**inputs):
    BC = B_FULL // N_CORES
    if BC not in _prog_cache:
        _prog_cache[BC] = build_program(BC)
    nc = _prog_cache[BC]
    in_maps = []
    for c in range(N_CORES):
        m = {}
        for name, shape, _ in input_specs_for(BC):
            arr = np.ascontiguousarray(np.asarray(inputs[name], dtype=np.float32))
            if name in SHARDED:
                arr = arr[c * BC:(c + 1) * BC]
            m[name] = np.ascontiguousarray(arr)
        in_maps.append(m)
    res = run_bass_kernel_spmd(nc, in_maps, core_ids=list(range(N_CORES)))
    return np.concatenate([r["out"] for r in res.results], axis=0)

